# revision 1
# baseline (speedup 1.0000x reference)
"""Causal multi-head attention (B=4, T=2048, C=1024, H=16) on 8 Trainium2 cores.

Sharding: core c handles batch b = c//2 and heads h0..h0+7 with h0 = (c%2)*8.
Each core computes QKV projection for its head slice, causal attention for its
8 heads, and a partial output projection. Host sums the two partials per batch
and adds the bias terms.

All matmuls run as float32r (full-speed fp32 on the PE, ~2e-4 relative error).

Attention works in the S^T = K Q^T layout ([k, q], k on partitions) so that
softmax normalization needs no cross-partition reduction: an extra all-ones
column appended to V makes the AV matmul emit the softmax row-sums for free,
and the causal mask is added into PSUM with an identity-matmul of a -1e30
bias tile before the exp.
"""

import os
import sys
import numpy as np

sys.path.insert(0, "/opt/trn_rl_repo")

import concourse.bass as bass  # noqa: E402
import concourse.bacc as bacc  # noqa: E402
import concourse.mybir as mybir  # noqa: E402
from concourse.bass_utils import run_bass_kernel_spmd  # noqa: E402
from concourse.tile import TileContext  # noqa: E402

B, T, C, H = 4, 2048, 1024, 16
HD = C // H          # 64 head dim
HPC = 8              # heads per core
P = 128
NT = T // P          # 16 t-chunks of 128
NS = T // 512        # 4 q-strips of 512
KC = C // P          # 8 contraction chunks for QKV
CL = HPC * HD        # 512 local channels per section
F32 = mybir.dt.float32
F32R = mybir.dt.float32r
BF16 = mybir.dt.bfloat16
EXPF = mybir.ActivationFunctionType.Exp
MUL = mybir.AluOpType.mult

_CACHED = {}


def build_nc():
    nc = bacc.Bacc("TRN2", target_bir_lowering=False, debug=False)

    xt_d = nc.dram_tensor("xt", [C, T], F32R, kind="ExternalInput")
    wqk_d = nc.dram_tensor("wqk", [C, 2 * CL], F32R, kind="ExternalInput")
    wv_d = nc.dram_tensor("wv", [C, CL], F32R, kind="ExternalInput")
    wp_d = nc.dram_tensor("wp", [CL, C], F32R, kind="ExternalInput")
    bqk_d = nc.dram_tensor("bqk", [P, 8], F32, kind="ExternalInput")
    ident_d = nc.dram_tensor("ident", [P, P], BF16, kind="ExternalInput")
    maskb_d = nc.dram_tensor("maskb", [P, P], BF16, kind="ExternalInput")
    ones_d = nc.dram_tensor("ones", [P, NT * HPC], F32R, kind="ExternalInput")
    y_d = nc.dram_tensor("y", [T, C], F32, kind="ExternalOutput")

    xt_r = xt_d.ap().rearrange("(kc p) t -> p kc t", p=P)       # [128, 8, 2048]
    wqk_r = wqk_d.ap().rearrange("(kc p) c -> p kc c", p=P)     # [128, 8, 1024]
    wv_r = wv_d.ap().rearrange("(kc p) c -> p kc c", p=P)       # [128, 8, 512]
    wp_r = wp_d.ap().rearrange("(ct p) c -> p ct c", p=P)       # [128, 4, 1024]
    y_r = y_d.ap().rearrange("(tt p) c -> p tt c", p=P)         # [128, 16, 1024]

    SW = 256            # phase-A pass-1 (v) t-strip width
    SW2 = 512           # phase-A pass-2 (qk) t-strip width
    EH = HD + 1         # 65: head slot width in v (value cols + ones col)
    scale = float(HD) ** -0.5

    with TileContext(nc) as tc:
      with tc.tile_pool(name="const", bufs=1) as constp:
        with tc.tile_pool(name="qkv_big", bufs=1) as bigp:
            v_sb = bigp.tile([P, NT, HPC * EH], F32R)
            v_heads = v_sb[:].rearrange("p t (h e) -> p t h e", e=EH)
            qkT = bigp.tile([P, 8, T], F32R)  # c-tiles 0-3 = qT, 4-7 = kT

            # ---------------- Phase A: QKV projections ----------------
            # Single pass per 256-wide t-strip: V matmuls then qT/kT matmuls.
            # All DMAs are chunked per contraction block so the PE can start
            # as soon as the first chunks land; wqk chunks stream during the
            # early v work.
            with (
                tc.tile_pool(name="xts", bufs=3) as xtsp,
                tc.tile_pool(name="wqkv", bufs=1) as wqkvp,
                tc.tile_pool(name="ps_a", bufs=3, space="PSUM") as ps_a,
            ):
                wv_sb = wqkvp.tile([P, KC, CL], F32R)
                wqk_sb = wqkvp.tile([P, KC, 2 * CL], F32R)
                xts0 = xtsp.tile([P, KC, SW], F32R, tag="xts")
                for kc in range(KC):
                    nc.sync.dma_start(xts0[:, kc, :], xt_r[:, kc, 0:SW])
                    nc.sync.dma_start(wv_sb[:, kc, :], wv_r[:, kc, :])
                bqk = constp.tile([P, 8], F32)
                nc.sync.dma_start(bqk[:], bqk_d[:])
                for kc in range(KC):
                    nc.sync.dma_start(wqk_sb[:, kc, :], wqk_r[:, kc, :])
                ident = constp.tile([P, P], BF16)
                nc.sync.dma_start(ident[:], ident_d[:])
                maskb = constp.tile([P, P], BF16)
                nc.sync.dma_start(maskb[:], maskb_d[:])
                nc.sync.dma_start(v_heads[:, :, :, HD], ones_d[:])

                for ts in range(T // SW):
                    if ts == 0:
                        xts = xts0
                    else:
                        xts = xtsp.tile([P, KC, SW], F32R, tag="xts")
                        for kc in range(KC):
                            nc.sync.dma_start(
                                xts[:, kc, :],
                                xt_r[:, kc, ts * SW:(ts + 1) * SW])
                    # v part: [t, c] orientation
                    for tt in range(SW // P):
                        tch = ts * (SW // P) + tt
                        psv = ps_a.tile([P, CL], F32, tag="psa")
                        for kc in range(KC):
                            nc.tensor.matmul(
                                psv[:],
                                xts[:, kc, tt * P:(tt + 1) * P],
                                wv_sb[:, kc, :],
                                start=(kc == 0), stop=(kc == KC - 1),
                            )
                        nc.vector.tensor_copy(
                            v_heads[:, tch, :, 0:HD],
                            psv[:].rearrange("p (h d) -> p h d", d=HD),
                        )
                    # qT/kT part: [c, t] orientation
                    for ct in range(8):
                        psq = ps_a.tile([P, SW], F32, tag="psq")
                        for kc in range(KC):
                            nc.tensor.matmul(
                                psq[:],
                                wqk_sb[:, kc, ct * P:(ct + 1) * P],
                                xts[:, kc, :],
                                start=(kc == 0), stop=(kc == KC - 1),
                            )
                        nc.vector.tensor_scalar_add(
                            qkT[:, ct, ts * SW:(ts + 1) * SW],
                            psq[:],
                            bqk[:, ct:ct + 1],
                        )

            # ---------------- Phase B: attention + overlapped projection ----
            # Strip-major over q; heads run in pairs sharing a qkT c-tile
            # (rows 0-63 / 64-127 -> different PE row groups). The AV matmuls
            # lag the score matmuls by LAG k-tiles so the PE never waits on
            # the exp. Each pair is normalized right after its AV finishes
            # (fast-reciprocal + gpsimd row broadcast), and the previous
            # strip's output projection is interleaved into the current
            # strip's attention.
            with (
                tc.tile_pool(name="attnT_p", bufs=1) as attnTp,
                tc.tile_pool(name="proj", bufs=1) as projp,
                tc.tile_pool(name="ystage", bufs=3) as ystagep,
            ):
                attnT = attnTp.tile([P, 4, T], F32R)
                wp_sb = projp.tile([P, 4, C], F32R)

                with (
                    tc.tile_pool(name="u_pool", bufs=5) as up,
                    tc.tile_pool(name="attn_small", bufs=2) as smallp,
                    tc.tile_pool(name="attn_one", bufs=1) as small1p,
                    tc.tile_pool(name="o_un", bufs=5) as ounp,
                    tc.tile_pool(name="ps_s", bufs=2, space="PSUM") as ps_s,
                    tc.tile_pool(name="ps_o", bufs=2, space="PSUM") as ps_o,
                ):
                    LAG = 2

                    def proj_tiles(tt, ps_y):
                        for co in range(2):
                            psy = ps_y.tile([P, 512], F32, tag="psy")
                            for ct in range(4):
                                nc.tensor.matmul(
                                    psy[:],
                                    attnT[:, ct, tt * P:(tt + 1) * P],
                                    wp_sb[:, ct, co * 512:(co + 1) * 512],
                                    start=(ct == 0), stop=(ct == 3),
                                )
                            yt = ystagep.tile([P, 512], F32, tag="yt")
                            nc.vector.tensor_copy(yt[:], psy[:])
                            nc.sync.dma_start(
                                y_r[:, tt, co * 512:(co + 1) * 512], yt[:])

                    nc.sync.dma_start(wp_sb[:], wp_r)
                    from contextlib import ExitStack
                    _stk = ExitStack()
                    ps_x = _stk.enter_context(
                        tc.tile_pool(name="ps_x", bufs=1, space="PSUM"))
                    ps_y = None
                    _nps = 0
                    for qj in range(NS):
                        if qj == 1:
                            _stk.close()  # release strip-0 extra psum
                            _stk = ExitStack()
                            ps_x = None
                            ps_y = _stk.enter_context(
                                tc.tile_pool(name="ps_y", bufs=2,
                                             space="PSUM"))
                        nk = 4 * (qj + 1)
                        for pr in range(4):  # head pair (2pr, 2pr+1)
                            qct, kct = pr, 4 + pr
                            psoA = ps_o.tile([EH, 512], F32, tag="ps_o")
                            psoB = ps_o.tile([EH, 512], F32, tag="ps_o")
                            u_ring = {}
                            for step in range(nk + LAG):
                                if step < nk:
                                    kt = step
                                    # columns < q0 of a diagonal tile are
                                    # fully masked: skip them in the scores,
                                    # exp, and AV; only the [128,128] block
                                    # at the diagonal needs the -1e30 mask.
                                    q0 = max(0, kt * P - qj * 512)
                                    diag = kt >= 4 * qj
                                    if ps_x is not None and _nps % 3 == 2:
                                        ps = ps_x.tile([P, 2, 512], F32,
                                                       tag="ps_x")
                                    else:
                                        ps = ps_s.tile([P, 2, 512], F32,
                                                       tag="ps_s")
                                    _nps += 1
                                    u = up.tile([P, 2, 512], F32R, tag="u")
                                    u_ring[kt] = u
                                    for hh in range(2):
                                        hp = hh * HD
                                        nc.tensor.matmul(
                                            ps[:, hh, q0:512],
                                            qkT[hp:hp + HD, kct,
                                                kt * P:(kt + 1) * P],
                                            qkT[hp:hp + HD, qct,
                                                qj * 512 + q0:
                                                (qj + 1) * 512],
                                            start=True, stop=not diag,
                                        )
                                    if diag:
                                        for hh in range(2):
                                            nc.tensor.matmul(
                                                ps[:, hh, q0:q0 + P],
                                                ident[:],
                                                maskb[:],
                                                start=False, stop=True,
                                            )
                                    nc.scalar.activation(
                                        u[:, :, q0:512], ps[:, :, q0:512],
                                        EXPF, scale=scale,
                                    )
                                if step >= LAG:
                                    kt = step - LAG
                                    u = u_ring.pop(kt)
                                    q0 = max(0, kt * P - qj * 512)
                                    last = kt == nk - 1
                                    nc.tensor.matmul(
                                        psoA[0:EH, q0:512],
                                        v_sb[:, kt,
                                             (2 * pr) * EH:(2 * pr + 1) * EH],
                                        u[:, 0, q0:512],
                                        start=(kt == 0), stop=last,
                                    )
                                    nc.tensor.matmul(
                                        psoB[0:EH, q0:512],
                                        v_sb[:, kt,
                                             (2 * pr + 1) * EH:
                                             (2 * pr + 2) * EH],
                                        u[:, 1, q0:512],
                                        start=(kt == 0), stop=last,
                                    )
                            # per-pair normalize: rowsums -> 1/x -> broadcast
                            rs = small1p.tile([2, 512], F32, tag="rs")
                            o_pair = []
                            for hh in range(2):
                                oun = ounp.tile([EH, 512], F32, tag="oun")
                                nc.vector.tensor_copy(
                                    oun[:], psoA[:] if hh == 0 else psoB[:])
                                nc.sync.dma_start(
                                    rs[hh:hh + 1, :], oun[HD:EH, :])
                                o_pair.append(oun)
                            rc = small1p.tile([2, 512], F32, tag="rc")
                            nc.vector.reciprocal(rc[:], rs[:])
                            for hh in (1, 0):
                                h = 2 * pr + hh
                                if hh == 0:
                                    # head A's recip row is already at
                                    # partition 0 -- broadcast it directly
                                    src_row = rc[0:1, :]
                                else:
                                    rc0 = smallp.tile(
                                        [1, 512], F32, tag="rc0")
                                    nc.sync.dma_start(
                                        rc0[:], rc[hh:hh + 1, :])
                                    src_row = rc0[:]
                                bc = smallp.tile([HD, 512], F32, tag="bc")
                                nc.gpsimd.partition_broadcast(
                                    bc[:], src_row)
                                if hh == 0:
                                    nc.vector.tensor_tensor(
                                        attnT[0:HD, pr,
                                              qj * 512:(qj + 1) * 512],
                                        o_pair[0][0:HD, :], bc[:], MUL,
                                    )
                                else:
                                    odd = smallp.tile(
                                        [HD, 512], F32R, tag="odd")
                                    nc.vector.tensor_tensor(
                                        odd[:], o_pair[1][0:HD, :],
                                        bc[:], MUL,
                                    )
                                    nc.sync.dma_start(
                                        attnT[HD:P, pr,
                                              qj * 512:(qj + 1) * 512],
                                        odd[:],
                                    )
                            # interleave previous strip's projection work
                            if qj > 0:
                                proj_tiles(4 * (qj - 1) + pr, ps_y)
                    _stk.close()
                # last strip's projection with a wide psum pool (the
                # attention pools are closed by now) so its first three
                # accumulation steps overlap the final normalize chain
                with tc.tile_pool(name="ps_f", bufs=8, space="PSUM") as ps_f:
                    chains = []
                    for tt in range(4 * (NS - 1), 4 * NS):
                        for co in range(2):
                            chains.append(
                                (tt, co,
                                 ps_f.tile([P, 512], F32, tag="psf",
                                           name=f"psf_{tt}_{co}")))
                    # stage-major so every chain's first three accumulation
                    # steps can run while the last head pair normalizes
                    for ct in range(4):
                        for tt, co, psy in chains:
                            nc.tensor.matmul(
                                psy[:],
                                attnT[:, ct, tt * P:(tt + 1) * P],
                                wp_sb[:, ct, co * 512:(co + 1) * 512],
                                start=(ct == 0), stop=(ct == 3),
                            )
                    for i, (tt, co, psy) in enumerate(chains):
                        yt = ystagep.tile([P, 512], F32, tag="yt")
                        if i % 2 == 0:
                            nc.scalar.copy(yt[:], psy[:])
                        else:
                            nc.vector.tensor_copy(yt[:], psy[:])
                        nc.sync.dma_start(
                            y_r[:, tt, co * 512:(co + 1) * 512], yt[:])
    nc.compile()
    return nc


def _host_consts():
    import ml_dtypes
    i_idx = np.arange(P, dtype=np.float32)[:, None]
    j_idx = np.arange(P, dtype=np.float32)[None, :]
    maskb = np.where(j_idx - i_idx >= 0, 0.0, -1e30).astype(ml_dtypes.bfloat16)
    ident = np.eye(P, dtype=ml_dtypes.bfloat16)
    ones = np.ones((P, NT * HPC), dtype=np.float32)
    return ident, maskb, ones


def make_in_maps(x, w_attn, b_attn, w_proj):
    ident, maskb, ones = _host_consts()
    in_maps = []
    for c in range(8):
        b = c // 2
        h0 = (c % 2) * HPC
        qcols = slice(h0 * HD, h0 * HD + CL)
        kcols = slice(C + h0 * HD, C + h0 * HD + CL)
        vcols = slice(2 * C + h0 * HD, 2 * C + h0 * HD + CL)
        wqk = np.concatenate([w_attn[:, qcols], w_attn[:, kcols]], axis=1)
        bqk = np.concatenate([b_attn[qcols], b_attn[kcols]]).reshape(8, P).T
        in_maps.append({
            "xt": np.ascontiguousarray(x[b].T),
            "wqk": np.ascontiguousarray(wqk),
            "wv": np.ascontiguousarray(w_attn[:, vcols]),
            "wp": np.ascontiguousarray(w_proj[h0 * HD:h0 * HD + CL, :]),
            "bqk": np.ascontiguousarray(bqk),
            "ident": ident,
            "maskb": maskb,
            "ones": ones,
        })
    return in_maps


def _get_runner():
    """Build the SPMD executor once: a cached jax.jit over 8 cores.

    Mirrors bass2jax.run_bass_via_pjrt but hoists the jit so repeated
    kernel() calls reuse the compiled executable.
    """
    if "runner" in _CACHED:
        return _CACHED["runner"]
    import jax
    import jax.numpy as jnp
    from jax.sharding import Mesh, PartitionSpec
    from jax.experimental.shard_map import shard_map
    from concourse import bass2jax
    import concourse.mybir as mybir_

    nc = _CACHED.get("nc")
    if nc is None:
        nc = _CACHED["nc"] = build_nc()
    bass2jax.install_neuronx_cc_hook()

    partition_name = (nc.partition_id_tensor.name
                      if nc.partition_id_tensor else None)
    in_names, out_names, out_avals, zero_shapes = [], [], [], []
    for alloc in nc.m.functions[0].allocations:
        if not isinstance(alloc, mybir_.MemoryLocationSet):
            continue
        name = alloc.memorylocations[0].name
        if alloc.kind == "ExternalInput":
            if name != partition_name:
                in_names.append(name)
        elif alloc.kind == "ExternalOutput":
            shape = tuple(alloc.tensor_shape)
            dtype = mybir_.dt.np(alloc.dtype)
            out_names.append(name)
            out_avals.append(jax.core.ShapedArray(shape, dtype))
            zero_shapes.append((shape, dtype))
    n_params = len(in_names)
    n_outs = len(out_names)
    all_names = in_names + out_names
    if partition_name is not None:
        all_names = all_names + [partition_name]

    def _body(*args):
        operands = list(args)
        if partition_name is not None:
            operands.append(bass2jax.partition_id_tensor())
        outs = bass2jax._bass_exec_p.bind(
            *operands,
            out_avals=tuple(out_avals),
            in_names=tuple(all_names),
            out_names=tuple(out_names),
            lowering_input_output_aliases=(),
            sim_require_finite=True,
            sim_require_nnan=True,
            nc=nc,
        )
        return tuple(outs)

    devices = jax.devices()[:8]
    mesh = Mesh(np.asarray(devices), ("core",))
    in_specs = (PartitionSpec("core"),) * (n_params + n_outs)
    out_specs = (PartitionSpec("core"),) * n_outs
    donate = tuple(range(n_params, n_params + n_outs))
    sharded = jax.jit(
        shard_map(_body, mesh=mesh, in_specs=in_specs, out_specs=out_specs,
                  check_rep=False),
        donate_argnums=donate, keep_unused=True,
    )

    def run(in_maps):
        concat_in = [
            np.concatenate([np.asarray(in_maps[c][nm]) for c in range(8)],
                           axis=0)
            for nm in in_names
        ]
        concat_zeros = [
            np.zeros((8 * s[0], *s[1:]), dt) for (s, dt) in zero_shapes
        ]
        out_arrs = sharded(*concat_in, *concat_zeros)
        return [
            {nm: np.asarray(out_arrs[i]).reshape(8, *out_avals[i].shape)[c]
             for i, nm in enumerate(out_names)}
            for c in range(8)
        ]

    _CACHED["runner"] = run
    return run


def kernel(x, w_attn, b_attn, w_proj, b_proj):
    x = np.asarray(x, dtype=np.float32)
    w_attn = np.asarray(w_attn, dtype=np.float32)
    b_attn = np.asarray(b_attn, dtype=np.float32)
    w_proj = np.asarray(w_proj, dtype=np.float32)
    b_proj = np.asarray(b_proj, dtype=np.float32)

    in_maps = make_in_maps(x, w_attn, b_attn, w_proj)
    try:
        run = _get_runner()
        results = run(in_maps)
    except Exception:
        # fallback: the stock SPMD runner (slower per call, same result)
        if "nc" not in _CACHED:
            _CACHED["nc"] = build_nc()
        res = run_bass_kernel_spmd(
            _CACHED["nc"], in_maps, core_ids=list(range(8)))
        results = res.results

    # v-bias contribution: probs rows sum to 1, so attn += 1 * b_v^T, and
    # (1 b_v^T) @ w_proj = row vector b_v @ w_proj added to every position.
    extra = b_attn[2 * C:] @ w_proj + b_proj  # [C]
    out = np.empty((B, T, C), dtype=np.float32)
    for b in range(B):
        out[b] = results[2 * b]["y"] + results[2 * b + 1]["y"] + extra
    return out



# revision 5
# speedup vs baseline: 1.0482x; 1.0482x over previous
"""Causal multi-head attention (B=4, T=2048, C=1024, H=16) on 8 Trainium2 cores.

Sharding: core c handles batch b = c//2 and heads h0..h0+7 with h0 = (c%2)*8.
Each core computes QKV projection for its head slice, causal attention for its
8 heads, and a partial output projection. Host sums the two partials per batch
and adds the bias terms.

All matmuls run as float32r (full-speed fp32 on the PE, ~2e-4 relative error).

Attention works in the S^T = K Q^T layout ([k, q], k on partitions) so that
softmax normalization needs no cross-partition reduction: an extra all-ones
column appended to V makes the AV matmul emit the softmax row-sums for free,
and the causal mask is added into PSUM with an identity-matmul of a -1e30
bias tile before the exp.
"""

import os
import sys
import numpy as np

sys.path.insert(0, "/opt/trn_rl_repo")

import concourse.bass as bass  # noqa: E402
import concourse.bacc as bacc  # noqa: E402
import concourse.mybir as mybir  # noqa: E402
from concourse.bass_utils import run_bass_kernel_spmd  # noqa: E402
from concourse.tile import TileContext  # noqa: E402

B, T, C, H = 4, 2048, 1024, 16
HD = C // H          # 64 head dim
HPC = 8              # heads per core
P = 128
NT = T // P          # 16 t-chunks of 128
NS = T // 512        # 4 q-strips of 512
KC = C // P          # 8 contraction chunks for QKV
CL = HPC * HD        # 512 local channels per section
F32 = mybir.dt.float32
F32R = mybir.dt.float32r
BF16 = mybir.dt.bfloat16
EXPF = mybir.ActivationFunctionType.Exp
MUL = mybir.AluOpType.mult

_CACHED = {}


def build_nc():
    nc = bacc.Bacc("TRN2", target_bir_lowering=False, debug=False)

    xt_d = nc.dram_tensor("xt", [C, T], BF16, kind="ExternalInput")
    wqk_d = nc.dram_tensor("wqk", [C, 2 * CL], BF16, kind="ExternalInput")
    wv_d = nc.dram_tensor("wv", [C, CL], BF16, kind="ExternalInput")
    wp_d = nc.dram_tensor("wp", [CL, C], BF16, kind="ExternalInput")
    bqk_d = nc.dram_tensor("bqk", [P, 8], F32, kind="ExternalInput")
    ident_d = nc.dram_tensor("ident", [P, P], BF16, kind="ExternalInput")
    maskb_d = nc.dram_tensor("maskb", [P, P], BF16, kind="ExternalInput")
    ones_d = nc.dram_tensor("ones", [P, NT * HPC], BF16, kind="ExternalInput")
    y_d = nc.dram_tensor("y", [T, C], F32, kind="ExternalOutput")

    xt_r = xt_d.ap().rearrange("(kc p) t -> p kc t", p=P)       # [128, 8, 2048]
    wqk_r = wqk_d.ap().rearrange("(kc p) c -> p kc c", p=P)     # [128, 8, 1024]
    wv_r = wv_d.ap().rearrange("(kc p) c -> p kc c", p=P)       # [128, 8, 512]
    wp_r = wp_d.ap().rearrange("(ct p) c -> p ct c", p=P)       # [128, 4, 1024]
    y_r = y_d.ap().rearrange("(tt p) c -> p tt c", p=P)         # [128, 16, 1024]

    SW = 256            # phase-A pass-1 (v) t-strip width
    SW2 = 512           # phase-A pass-2 (qk) t-strip width
    EH = HD + 1         # 65: head slot width in v (value cols + ones col)
    scale = float(HD) ** -0.5

    with TileContext(nc) as tc:
      with tc.tile_pool(name="const", bufs=1) as constp:
        with tc.tile_pool(name="qkv_big", bufs=1) as bigp:
            v_sb = bigp.tile([P, NT, HPC * EH], BF16)
            v_heads = v_sb[:].rearrange("p t (h e) -> p t h e", e=EH)
            qkT = bigp.tile([P, 8, T], BF16)  # c-tiles 0-3 = qT, 4-7 = kT

            # ---------------- Phase A: QKV projections ----------------
            # Single pass per 256-wide t-strip: V matmuls then qT/kT matmuls.
            # All DMAs are chunked per contraction block so the PE can start
            # as soon as the first chunks land; wqk chunks stream during the
            # early v work.
            with (
                tc.tile_pool(name="xts", bufs=3) as xtsp,
                tc.tile_pool(name="wqkv", bufs=1) as wqkvp,
                tc.tile_pool(name="ps_a", bufs=3, space="PSUM") as ps_a,
            ):
                wv_sb = wqkvp.tile([P, KC, CL], BF16)
                wqk_sb = wqkvp.tile([P, KC, 2 * CL], BF16)
                xts0 = xtsp.tile([P, KC, SW], BF16, tag="xts")
                for kc in range(KC):
                    nc.sync.dma_start(xts0[:, kc, :], xt_r[:, kc, 0:SW])
                    nc.sync.dma_start(wv_sb[:, kc, :], wv_r[:, kc, :])
                bqk = constp.tile([P, 8], F32)
                nc.sync.dma_start(bqk[:], bqk_d[:])
                for kc in range(KC):
                    nc.sync.dma_start(wqk_sb[:, kc, :], wqk_r[:, kc, :])
                ident = constp.tile([P, P], BF16)
                nc.sync.dma_start(ident[:], ident_d[:])
                maskb = constp.tile([P, P], BF16)
                nc.sync.dma_start(maskb[:], maskb_d[:])
                nc.sync.dma_start(v_heads[:, :, :, HD], ones_d[:])

                for ts in range(T // SW):
                    if ts == 0:
                        xts = xts0
                    else:
                        xts = xtsp.tile([P, KC, SW], BF16, tag="xts")
                        for kc in range(KC):
                            nc.sync.dma_start(
                                xts[:, kc, :],
                                xt_r[:, kc, ts * SW:(ts + 1) * SW])
                    # v part: [t, c] orientation
                    for tt in range(SW // P):
                        tch = ts * (SW // P) + tt
                        psv = ps_a.tile([P, CL], F32, tag="psa")
                        for kc in range(KC):
                            nc.tensor.matmul(
                                psv[:],
                                xts[:, kc, tt * P:(tt + 1) * P],
                                wv_sb[:, kc, :],
                                start=(kc == 0), stop=(kc == KC - 1),
                            )
                        nc.vector.tensor_copy(
                            v_heads[:, tch, :, 0:HD],
                            psv[:].rearrange("p (h d) -> p h d", d=HD),
                        )
                    # qT/kT part: [c, t] orientation
                    for ct in range(8):
                        psq = ps_a.tile([P, SW], F32, tag="psq")
                        for kc in range(KC):
                            nc.tensor.matmul(
                                psq[:],
                                wqk_sb[:, kc, ct * P:(ct + 1) * P],
                                xts[:, kc, :],
                                start=(kc == 0), stop=(kc == KC - 1),
                            )
                        nc.vector.tensor_scalar_add(
                            qkT[:, ct, ts * SW:(ts + 1) * SW],
                            psq[:],
                            bqk[:, ct:ct + 1],
                        )

            # ---------------- Phase B: attention + overlapped projection ----
            # Strip-major over q; heads run in pairs sharing a qkT c-tile
            # (rows 0-63 / 64-127 -> different PE row groups). The AV matmuls
            # lag the score matmuls by LAG k-tiles so the PE never waits on
            # the exp. Each pair is normalized right after its AV finishes
            # (fast-reciprocal + gpsimd row broadcast), and the previous
            # strip's output projection is interleaved into the current
            # strip's attention.
            with (
                tc.tile_pool(name="attnT_p", bufs=1) as attnTp,
                tc.tile_pool(name="proj", bufs=1) as projp,
                tc.tile_pool(name="ystage", bufs=3) as ystagep,
            ):
                attnT = attnTp.tile([P, 4, T], BF16)
                wp_sb = projp.tile([P, 4, C], BF16)

                with (
                    tc.tile_pool(name="u_pool", bufs=5) as up,
                    tc.tile_pool(name="attn_small", bufs=2) as smallp,
                    tc.tile_pool(name="attn_one", bufs=1) as small1p,
                    tc.tile_pool(name="o_un", bufs=5) as ounp,
                    tc.tile_pool(name="ps_s", bufs=2, space="PSUM") as ps_s,
                    tc.tile_pool(name="ps_o", bufs=2, space="PSUM") as ps_o,
                ):
                    LAG = 2

                    def proj_tiles(tt, ps_y):
                        for co in range(2):
                            psy = ps_y.tile([P, 512], F32, tag="psy")
                            for ct in range(4):
                                nc.tensor.matmul(
                                    psy[:],
                                    attnT[:, ct, tt * P:(tt + 1) * P],
                                    wp_sb[:, ct, co * 512:(co + 1) * 512],
                                    start=(ct == 0), stop=(ct == 3),
                                )
                            yt = ystagep.tile([P, 512], F32, tag="yt")
                            nc.vector.tensor_copy(yt[:], psy[:])
                            nc.sync.dma_start(
                                y_r[:, tt, co * 512:(co + 1) * 512], yt[:])

                    nc.sync.dma_start(wp_sb[:], wp_r)
                    from contextlib import ExitStack
                    _stk = ExitStack()
                    ps_x = _stk.enter_context(
                        tc.tile_pool(name="ps_x", bufs=1, space="PSUM"))
                    ps_y = None
                    _nps = 0
                    for qj in range(NS):
                        if qj == 1:
                            _stk.close()  # release strip-0 extra psum
                            _stk = ExitStack()
                            ps_x = None
                            ps_y = _stk.enter_context(
                                tc.tile_pool(name="ps_y", bufs=2,
                                             space="PSUM"))
                        nk = 4 * (qj + 1)
                        for pr in range(4):  # head pair (2pr, 2pr+1)
                            qct, kct = pr, 4 + pr
                            psoA = ps_o.tile([EH, 512], F32, tag="ps_o")
                            psoB = ps_o.tile([EH, 512], F32, tag="ps_o")
                            u_ring = {}
                            for step in range(nk + LAG):
                                if step < nk:
                                    kt = step
                                    # columns < q0 of a diagonal tile are
                                    # fully masked: skip them in the scores,
                                    # exp, and AV; only the [128,128] block
                                    # at the diagonal needs the -1e30 mask.
                                    q0 = max(0, kt * P - qj * 512)
                                    diag = kt >= 4 * qj
                                    if ps_x is not None and _nps % 3 == 2:
                                        ps = ps_x.tile([P, 2, 512], F32,
                                                       tag="ps_x")
                                    else:
                                        ps = ps_s.tile([P, 2, 512], F32,
                                                       tag="ps_s")
                                    _nps += 1
                                    u = up.tile([P, 2, 512], BF16, tag="u")
                                    u_ring[kt] = u
                                    for hh in range(2):
                                        hp = hh * HD
                                        nc.tensor.matmul(
                                            ps[:, hh, q0:512],
                                            qkT[hp:hp + HD, kct,
                                                kt * P:(kt + 1) * P],
                                            qkT[hp:hp + HD, qct,
                                                qj * 512 + q0:
                                                (qj + 1) * 512],
                                            start=True, stop=not diag,
                                        )
                                    if diag:
                                        for hh in range(2):
                                            nc.tensor.matmul(
                                                ps[:, hh, q0:q0 + P],
                                                ident[:],
                                                maskb[:],
                                                start=False, stop=True,
                                            )
                                    nc.scalar.activation(
                                        u[:, :, q0:512], ps[:, :, q0:512],
                                        EXPF, scale=scale,
                                    )
                                if step >= LAG:
                                    kt = step - LAG
                                    u = u_ring.pop(kt)
                                    q0 = max(0, kt * P - qj * 512)
                                    last = kt == nk - 1
                                    nc.tensor.matmul(
                                        psoA[0:EH, q0:512],
                                        v_sb[:, kt,
                                             (2 * pr) * EH:(2 * pr + 1) * EH],
                                        u[:, 0, q0:512],
                                        start=(kt == 0), stop=last,
                                    )
                                    nc.tensor.matmul(
                                        psoB[0:EH, q0:512],
                                        v_sb[:, kt,
                                             (2 * pr + 1) * EH:
                                             (2 * pr + 2) * EH],
                                        u[:, 1, q0:512],
                                        start=(kt == 0), stop=last,
                                    )
                            # per-pair normalize: rowsums -> 1/x -> broadcast
                            rs = small1p.tile([2, 512], F32, tag="rs")
                            o_pair = []
                            for hh in range(2):
                                oun = ounp.tile([EH, 512], F32, tag="oun")
                                nc.vector.tensor_copy(
                                    oun[:], psoA[:] if hh == 0 else psoB[:])
                                nc.sync.dma_start(
                                    rs[hh:hh + 1, :], oun[HD:EH, :])
                                o_pair.append(oun)
                            rc = small1p.tile([2, 512], F32, tag="rc")
                            nc.vector.reciprocal(rc[:], rs[:])
                            for hh in (1, 0):
                                h = 2 * pr + hh
                                if hh == 0:
                                    # head A's recip row is already at
                                    # partition 0 -- broadcast it directly
                                    src_row = rc[0:1, :]
                                else:
                                    rc0 = smallp.tile(
                                        [1, 512], F32, tag="rc0")
                                    nc.sync.dma_start(
                                        rc0[:], rc[hh:hh + 1, :])
                                    src_row = rc0[:]
                                bc = smallp.tile([HD, 512], F32, tag="bc")
                                nc.gpsimd.partition_broadcast(
                                    bc[:], src_row)
                                if hh == 0:
                                    nc.vector.tensor_tensor(
                                        attnT[0:HD, pr,
                                              qj * 512:(qj + 1) * 512],
                                        o_pair[0][0:HD, :], bc[:], MUL,
                                    )
                                else:
                                    odd = smallp.tile(
                                        [HD, 512], BF16, tag="odd")
                                    nc.vector.tensor_tensor(
                                        odd[:], o_pair[1][0:HD, :],
                                        bc[:], MUL,
                                    )
                                    nc.sync.dma_start(
                                        attnT[HD:P, pr,
                                              qj * 512:(qj + 1) * 512],
                                        odd[:],
                                    )
                            # interleave previous strip's projection work
                            if qj > 0:
                                proj_tiles(4 * (qj - 1) + pr, ps_y)
                    _stk.close()
                # last strip's projection with a wide psum pool (the
                # attention pools are closed by now) so its first three
                # accumulation steps overlap the final normalize chain
                with tc.tile_pool(name="ps_f", bufs=8, space="PSUM") as ps_f:
                    chains = []
                    for tt in range(4 * (NS - 1), 4 * NS):
                        for co in range(2):
                            chains.append(
                                (tt, co,
                                 ps_f.tile([P, 512], F32, tag="psf",
                                           name=f"psf_{tt}_{co}")))
                    # stage-major so every chain's first three accumulation
                    # steps can run while the last head pair normalizes
                    for ct in range(4):
                        for tt, co, psy in chains:
                            nc.tensor.matmul(
                                psy[:],
                                attnT[:, ct, tt * P:(tt + 1) * P],
                                wp_sb[:, ct, co * 512:(co + 1) * 512],
                                start=(ct == 0), stop=(ct == 3),
                            )
                    for i, (tt, co, psy) in enumerate(chains):
                        yt = ystagep.tile([P, 512], F32, tag="yt")
                        if i % 2 == 0:
                            nc.scalar.copy(yt[:], psy[:])
                        else:
                            nc.vector.tensor_copy(yt[:], psy[:])
                        nc.sync.dma_start(
                            y_r[:, tt, co * 512:(co + 1) * 512], yt[:])
    nc.compile()
    return nc


def _host_consts():
    import ml_dtypes
    i_idx = np.arange(P, dtype=np.float32)[:, None]
    j_idx = np.arange(P, dtype=np.float32)[None, :]
    maskb = np.where(j_idx - i_idx >= 0, 0.0, -1e30).astype(ml_dtypes.bfloat16)
    ident = np.eye(P, dtype=ml_dtypes.bfloat16)
    ones = np.ones((P, NT * HPC), dtype=ml_dtypes.bfloat16)
    return ident, maskb, ones


def make_in_maps(x, w_attn, b_attn, w_proj):
    import ml_dtypes
    bf16 = ml_dtypes.bfloat16
    ident, maskb, ones = _host_consts()
    in_maps = []
    for c in range(8):
        b = c // 2
        h0 = (c % 2) * HPC
        qcols = slice(h0 * HD, h0 * HD + CL)
        kcols = slice(C + h0 * HD, C + h0 * HD + CL)
        vcols = slice(2 * C + h0 * HD, 2 * C + h0 * HD + CL)
        wqk = np.concatenate([w_attn[:, qcols], w_attn[:, kcols]], axis=1)
        bqk = np.concatenate([b_attn[qcols], b_attn[kcols]]).reshape(8, P).T
        in_maps.append({
            "xt": np.ascontiguousarray(x[b].T.astype(bf16)),
            "wqk": np.ascontiguousarray(wqk.astype(bf16)),
            "wv": np.ascontiguousarray(w_attn[:, vcols].astype(bf16)),
            "wp": np.ascontiguousarray(
                w_proj[h0 * HD:h0 * HD + CL, :].astype(bf16)),
            "bqk": np.ascontiguousarray(bqk),
            "ident": ident,
            "maskb": maskb,
            "ones": ones,
        })
    return in_maps


def _get_runner():
    """Build the SPMD executor once: a cached jax.jit over 8 cores.

    Mirrors bass2jax.run_bass_via_pjrt but hoists the jit so repeated
    kernel() calls reuse the compiled executable.
    """
    if "runner" in _CACHED:
        return _CACHED["runner"]
    import jax
    import jax.numpy as jnp
    from jax.sharding import Mesh, PartitionSpec
    from jax.experimental.shard_map import shard_map
    from concourse import bass2jax
    import concourse.mybir as mybir_

    nc = _CACHED.get("nc")
    if nc is None:
        nc = _CACHED["nc"] = build_nc()
    bass2jax.install_neuronx_cc_hook()

    partition_name = (nc.partition_id_tensor.name
                      if nc.partition_id_tensor else None)
    in_names, out_names, out_avals, zero_shapes = [], [], [], []
    for alloc in nc.m.functions[0].allocations:
        if not isinstance(alloc, mybir_.MemoryLocationSet):
            continue
        name = alloc.memorylocations[0].name
        if alloc.kind == "ExternalInput":
            if name != partition_name:
                in_names.append(name)
        elif alloc.kind == "ExternalOutput":
            shape = tuple(alloc.tensor_shape)
            dtype = mybir_.dt.np(alloc.dtype)
            out_names.append(name)
            out_avals.append(jax.core.ShapedArray(shape, dtype))
            zero_shapes.append((shape, dtype))
    n_params = len(in_names)
    n_outs = len(out_names)
    all_names = in_names + out_names
    if partition_name is not None:
        all_names = all_names + [partition_name]

    def _body(*args):
        operands = list(args)
        if partition_name is not None:
            operands.append(bass2jax.partition_id_tensor())
        outs = bass2jax._bass_exec_p.bind(
            *operands,
            out_avals=tuple(out_avals),
            in_names=tuple(all_names),
            out_names=tuple(out_names),
            lowering_input_output_aliases=(),
            sim_require_finite=True,
            sim_require_nnan=True,
            nc=nc,
        )
        return tuple(outs)

    devices = jax.devices()[:8]
    mesh = Mesh(np.asarray(devices), ("core",))
    in_specs = (PartitionSpec("core"),) * (n_params + n_outs)
    out_specs = (PartitionSpec("core"),) * n_outs
    donate = tuple(range(n_params, n_params + n_outs))
    sharded = jax.jit(
        shard_map(_body, mesh=mesh, in_specs=in_specs, out_specs=out_specs,
                  check_rep=False),
        donate_argnums=donate, keep_unused=True,
    )

    def run(in_maps):
        concat_in = [
            np.concatenate([np.asarray(in_maps[c][nm]) for c in range(8)],
                           axis=0)
            for nm in in_names
        ]
        concat_zeros = [
            np.zeros((8 * s[0], *s[1:]), dt) for (s, dt) in zero_shapes
        ]
        out_arrs = sharded(*concat_in, *concat_zeros)
        return [
            {nm: np.asarray(out_arrs[i]).reshape(8, *out_avals[i].shape)[c]
             for i, nm in enumerate(out_names)}
            for c in range(8)
        ]

    _CACHED["runner"] = run
    return run


def kernel(x, w_attn, b_attn, w_proj, b_proj):
    x = np.asarray(x, dtype=np.float32)
    w_attn = np.asarray(w_attn, dtype=np.float32)
    b_attn = np.asarray(b_attn, dtype=np.float32)
    w_proj = np.asarray(w_proj, dtype=np.float32)
    b_proj = np.asarray(b_proj, dtype=np.float32)

    in_maps = make_in_maps(x, w_attn, b_attn, w_proj)
    try:
        run = _get_runner()
        results = run(in_maps)
    except Exception:
        # fallback: the stock SPMD runner (slower per call, same result)
        if "nc" not in _CACHED:
            _CACHED["nc"] = build_nc()
        res = run_bass_kernel_spmd(
            _CACHED["nc"], in_maps, core_ids=list(range(8)))
        results = res.results

    # v-bias contribution: probs rows sum to 1, so attn += 1 * b_v^T, and
    # (1 b_v^T) @ w_proj = row vector b_v @ w_proj added to every position.
    extra = b_attn[2 * C:] @ w_proj + b_proj  # [C]
    out = np.empty((B, T, C), dtype=np.float32)
    for b in range(B):
        out[b] = results[2 * b]["y"] + results[2 * b + 1]["y"] + extra
    return out



# revision 21
# speedup vs baseline: 1.1152x; 1.0639x over previous
"""Causal multi-head attention (B=4, T=2048, C=1024, H=16) on 8 Trainium2 cores.

Sharding: core c handles batch b = c//2 and heads h0..h0+7 with h0 = (c%2)*8.
Each core computes QKV projection for its head slice, causal attention for its
8 heads, and a partial output projection. Host sums the two partials per batch
and adds the bias terms.

All matmuls run as float32r (full-speed fp32 on the PE, ~2e-4 relative error).

Attention works in the S^T = K Q^T layout ([k, q], k on partitions) so that
softmax normalization needs no cross-partition reduction: an extra all-ones
column appended to V makes the AV matmul emit the softmax row-sums for free,
and the causal mask is added into PSUM with an identity-matmul of a -1e30
bias tile before the exp.
"""

import os
import sys
import numpy as np

sys.path.insert(0, "/opt/trn_rl_repo")

import concourse.bass as bass  # noqa: E402
import concourse.bacc as bacc  # noqa: E402
import concourse.mybir as mybir  # noqa: E402
from concourse.bass_utils import run_bass_kernel_spmd  # noqa: E402
from concourse.tile import TileContext  # noqa: E402

B, T, C, H = 4, 2048, 1024, 16
HD = C // H          # 64 head dim
HPC = 8              # heads per core
P = 128
NT = T // P          # 16 t-chunks of 128
NS = T // 512        # 4 q-strips of 512
KC = C // P          # 8 contraction chunks for QKV
CL = HPC * HD        # 512 local channels per section
F32 = mybir.dt.float32
F32R = mybir.dt.float32r
BF16 = mybir.dt.bfloat16
EXPF = mybir.ActivationFunctionType.Exp
MUL = mybir.AluOpType.mult

_CACHED = {}


def build_nc():
    nc = bacc.Bacc("TRN2", target_bir_lowering=False, debug=False)

    xt_d = nc.dram_tensor("xt", [C, T], BF16, kind="ExternalInput")
    wqk_d = nc.dram_tensor("wqk", [C, 2 * CL], BF16, kind="ExternalInput")
    wv_d = nc.dram_tensor("wv", [C, CL], BF16, kind="ExternalInput")
    wp_d = nc.dram_tensor("wp", [CL, C], BF16, kind="ExternalInput")
    bqk_d = nc.dram_tensor("bqk", [P, 8], F32, kind="ExternalInput")
    ident_d = nc.dram_tensor("ident", [P, P], BF16, kind="ExternalInput")
    maskb_d = nc.dram_tensor("maskb", [P, P], BF16, kind="ExternalInput")
    y_d = nc.dram_tensor("y", [T, C], BF16, kind="ExternalOutput")

    xt_r = xt_d.ap().rearrange("(kc p) t -> p kc t", p=P)       # [128, 8, 2048]
    wqk_r = wqk_d.ap().rearrange("(kc p) c -> p kc c", p=P)     # [128, 8, 1024]
    wv_r = wv_d.ap().rearrange("(kc p) c -> p kc c", p=P)       # [128, 8, 512]
    wp_r = wp_d.ap().rearrange("(ct p) c -> p ct c", p=P)       # [128, 4, 1024]
    y_r = y_d.ap().rearrange("(tt p) c -> p tt c", p=P)         # [128, 16, 1024]

    SW = 256            # phase-A pass-1 (v) t-strip width
    SW2 = 512           # phase-A pass-2 (qk) t-strip width
    EH = HD + 1         # 65: head slot width in v (value cols + ones col)
    scale = float(HD) ** -0.5

    with TileContext(nc) as tc:
      with tc.tile_pool(name="const", bufs=1) as constp:
        with tc.tile_pool(name="qkv_big", bufs=1) as bigp:
            v_sb = bigp.tile([P, NT, HPC * EH], BF16)
            v_heads = v_sb[:].rearrange("p t (h e) -> p t h e", e=EH)
            qkT = bigp.tile([P, 8, T], BF16)  # c-tiles 0-3 = qT, 4-7 = kT

            # ---------------- Phase A: QKV projections ----------------
            # Single pass per 256-wide t-strip: V matmuls then qT/kT matmuls.
            # All DMAs are chunked per contraction block so the PE can start
            # as soon as the first chunks land; wqk chunks stream during the
            # early v work.
            with (
                tc.tile_pool(name="xts", bufs=4) as xtsp,
                tc.tile_pool(name="wqkv", bufs=1) as wqkvp,
                tc.tile_pool(name="ps_a", bufs=3, space="PSUM") as ps_a,
            ):
                wv_sb = wqkvp.tile([P, KC, CL], BF16)
                wqk_sb = wqkvp.tile([P, KC, 2 * CL], BF16)
                xts0 = xtsp.tile([P, KC, SW], BF16, tag="xts")
                for kc in range(KC):
                    nc.sync.dma_start(xts0[:, kc, :], xt_r[:, kc, 0:SW])
                    nc.sync.dma_start(wv_sb[:, kc, :], wv_r[:, kc, :])
                bqk = constp.tile([P, 8], F32)
                nc.sync.dma_start(bqk[:], bqk_d[:])
                for kc in range(KC):
                    nc.sync.dma_start(wqk_sb[:, kc, :], wqk_r[:, kc, :])
                ident = constp.tile([P, P], BF16)
                maskb = constp.tile([P, P], BF16)

                for ts in range(T // SW):
                    if ts == 0:
                        xts = xts0
                    else:
                        xts = xtsp.tile([P, KC, SW], BF16, tag="xts")
                        nc.sync.dma_start(
                            xts[:], xt_r[:, :, ts * SW:(ts + 1) * SW])
                    # v part: [t, c] orientation
                    for tt in range(SW // P):
                        tch = ts * (SW // P) + tt
                        psv = ps_a.tile([P, CL], F32, tag="psa")
                        for kc in range(KC):
                            nc.tensor.matmul(
                                psv[:],
                                xts[:, kc, tt * P:(tt + 1) * P],
                                wv_sb[:, kc, :],
                                start=(kc == 0), stop=(kc == KC - 1),
                            )
                        nc.vector.tensor_copy(
                            v_heads[:, tch, :, 0:HD],
                            psv[:].rearrange("p (h d) -> p h d", d=HD),
                        )
                    # qT/kT part: [c, t] orientation
                    for ct in range(8):
                        psq = ps_a.tile([P, SW], F32, tag="psq")
                        for kc in range(KC):
                            nc.tensor.matmul(
                                psq[:],
                                wqk_sb[:, kc, ct * P:(ct + 1) * P],
                                xts[:, kc, :],
                                start=(kc == 0), stop=(kc == KC - 1),
                            )
                        nc.vector.tensor_scalar_add(
                            qkT[:, ct, ts * SW:(ts + 1) * SW],
                            psq[:],
                            bqk[:, ct:ct + 1],
                        )
                    if ts == 0:
                        # phase-B constants ride behind the early x strips;
                        # the ones column is a memset, not a DMA scatter
                        nc.sync.dma_start(ident[:], ident_d[:])
                        nc.sync.dma_start(maskb[:], maskb_d[:])
                        nc.vector.memset(v_heads[:, :, :, HD], 1.0)

            # ---------------- Phase B: attention + overlapped projection ----
            # Strip-major over q; heads run in pairs sharing a qkT c-tile
            # (rows 0-63 / 64-127 -> different PE row groups). The AV matmuls
            # lag the score matmuls by LAG k-tiles so the PE never waits on
            # the exp. Each pair is normalized right after its AV finishes
            # (fast-reciprocal + gpsimd row broadcast), and the previous
            # strip's output projection is interleaved into the current
            # strip's attention.
            with (
                tc.tile_pool(name="attnT_p", bufs=1) as attnTp,
                tc.tile_pool(name="proj", bufs=1) as projp,
                tc.tile_pool(name="ystage", bufs=4) as ystagep,
            ):
                attnT = attnTp.tile([P, 4, T], BF16)
                wp_sb = projp.tile([P, 4, C], BF16)

                with (
                    tc.tile_pool(name="u_pool", bufs=6) as up,
                    tc.tile_pool(name="attn_small", bufs=2) as smallp,
                    tc.tile_pool(name="attn_one", bufs=1) as small1p,
                    tc.tile_pool(name="o_un", bufs=5) as ounp,
                    tc.tile_pool(name="ps_s", bufs=2, space="PSUM") as ps_s,
                    tc.tile_pool(name="ps_o", bufs=2, space="PSUM") as ps_o,
                ):
                    LAG = 3

                    def proj_tiles(tt, ps_y):
                        for co in range(2):
                            psy = ps_y.tile([P, 512], F32, tag="psy")
                            for ct in range(4):
                                nc.tensor.matmul(
                                    psy[:],
                                    attnT[:, ct, tt * P:(tt + 1) * P],
                                    wp_sb[:, ct, co * 512:(co + 1) * 512],
                                    start=(ct == 0), stop=(ct == 3),
                                )
                            yt = ystagep.tile([P, 512], BF16, tag="yt")
                            nc.vector.tensor_copy(yt[:], psy[:])
                            nc.sync.dma_start(
                                y_r[:, tt, co * 512:(co + 1) * 512], yt[:])

                    nc.sync.dma_start(wp_sb[:], wp_r)
                    from contextlib import ExitStack
                    _stk = ExitStack()
                    ps_x = _stk.enter_context(
                        tc.tile_pool(name="ps_x", bufs=1, space="PSUM"))
                    ps_y = None
                    _nps = 0
                    for qj in range(NS):
                        if qj == 1:
                            _stk.close()  # release strip-0 extra psum
                            _stk = ExitStack()
                            ps_x = None
                            ps_y = _stk.enter_context(
                                tc.tile_pool(name="ps_y", bufs=2,
                                             space="PSUM"))
                        nk = 4 * (qj + 1)
                        for pr in range(4):  # head pair (2pr, 2pr+1)
                            qct, kct = pr, 4 + pr
                            psoA = ps_o.tile([EH, 512], F32, tag="ps_o")
                            psoB = ps_o.tile([EH, 512], F32, tag="ps_o")
                            u_ring = {}
                            for step in range(nk + LAG):
                                if step < nk:
                                    kt = step
                                    # columns < q0 of a diagonal tile are
                                    # fully masked: skip them in the scores,
                                    # exp, and AV; only the [128,128] block
                                    # at the diagonal needs the -1e30 mask.
                                    q0 = max(0, kt * P - qj * 512)
                                    diag = kt >= 4 * qj
                                    if ps_x is not None and _nps % 3 == 2:
                                        ps = ps_x.tile([P, 2, 512], F32,
                                                       tag="ps_x")
                                    else:
                                        ps = ps_s.tile([P, 2, 512], F32,
                                                       tag="ps_s")
                                    _nps += 1
                                    u = up.tile([P, 2, 512], BF16, tag="u")
                                    u_ring[kt] = u
                                    for hh in range(2):
                                        hp = hh * HD
                                        nc.tensor.matmul(
                                            ps[:, hh, q0:512],
                                            qkT[hp:hp + HD, kct,
                                                kt * P:(kt + 1) * P],
                                            qkT[hp:hp + HD, qct,
                                                qj * 512 + q0:
                                                (qj + 1) * 512],
                                            start=True, stop=not diag,
                                        )
                                    if diag:
                                        for hh in range(2):
                                            nc.tensor.matmul(
                                                ps[:, hh, q0:q0 + P],
                                                ident[:],
                                                maskb[:],
                                                start=False, stop=True,
                                            )
                                    nc.scalar.activation(
                                        u[:, :, q0:512], ps[:, :, q0:512],
                                        EXPF, scale=scale,
                                    )
                                if step >= LAG:
                                    kt = step - LAG
                                    u = u_ring.pop(kt)
                                    q0 = max(0, kt * P - qj * 512)
                                    last = kt == nk - 1
                                    nc.tensor.matmul(
                                        psoA[0:EH, q0:512],
                                        v_sb[:, kt,
                                             (2 * pr) * EH:(2 * pr + 1) * EH],
                                        u[:, 0, q0:512],
                                        start=(kt == 0), stop=last,
                                    )
                                    nc.tensor.matmul(
                                        psoB[0:EH, q0:512],
                                        v_sb[:, kt,
                                             (2 * pr + 1) * EH:
                                             (2 * pr + 2) * EH],
                                        u[:, 1, q0:512],
                                        start=(kt == 0), stop=last,
                                    )
                            # per-pair normalize: in-place reciprocal on
                            # the rowsum row (partition 64) + direct gpsimd
                            # broadcast from it -- no DMAs except the odd-head
                            # cross-partition store into attnT rows 64-127.
                            qsl = slice(qj * 512, (qj + 1) * 512)
                            # previous strip's projection: issued first so its
                            # psum-freeing copies sit ahead of the normalize
                            # chain in the DVE queue
                            if qj > 0:
                                proj_tiles(4 * (qj - 1) + pr, ps_y)
                            ounA = ounp.tile([EH, 512], F32, tag="oun")
                            nc.vector.tensor_copy(ounA[:], psoA[:])
                            ounB = ounp.tile([EH, 512], F32, tag="oun")
                            nc.vector.tensor_copy(ounB[:], psoB[:])
                            # cross-partition reciprocal p64 -> p0 (DVE allows
                            # partition remaps between aligned bases); the
                            # broadcast source must sit at partition 0 on hw
                            rcA = smallp.tile([1, 512], F32, tag="rc")
                            nc.vector.reciprocal(rcA[:], ounA[HD:EH, :])
                            rcB = smallp.tile([1, 512], F32, tag="rc")
                            nc.vector.reciprocal(rcB[:], ounB[HD:EH, :])
                            bcA = smallp.tile([HD, 512], F32, tag="bc")
                            nc.gpsimd.partition_broadcast(bcA[:], rcA[:])
                            bcB = smallp.tile([HD, 512], F32, tag="bc")
                            nc.gpsimd.partition_broadcast(bcB[:], rcB[:])
                            nc.vector.tensor_tensor(
                                attnT[0:HD, pr, qsl],
                                ounA[0:HD, :], bcA[:], MUL,
                            )
                            # odd head: DVE cross-partition write (0-63 ->
                            # 64-127), aligned partition bases
                            nc.vector.tensor_tensor(
                                attnT[HD:P, pr, qsl],
                                ounB[0:HD, :], bcB[:], MUL,
                            )
                    _stk.close()
                # last strip's projection with a wide psum pool (the
                # attention pools are closed by now) so its first three
                # accumulation steps overlap the final normalize chain
                with tc.tile_pool(name="ps_f", bufs=4, space="PSUM") as ps_f:
                    chains = []
                    for tt in range(4 * (NS - 1), 4 * NS):
                        chains.append(
                            (tt, ps_f.tile([P, 1024], F32, tag="psf",
                                           name=f"psf_{tt}")))
                    # stage-major so every chain's first three accumulation
                    # steps can run while the last head pair normalizes; the
                    # last stage goes chain-major so each chain's copy + DMA
                    # launches as soon as its accumulation closes
                    for ct in range(3):
                        for tt, psy in chains:
                            for co in range(2):
                                nc.tensor.matmul(
                                    psy[:, co * 512:(co + 1) * 512],
                                    attnT[:, ct, tt * P:(tt + 1) * P],
                                    wp_sb[:, ct, co * 512:(co + 1) * 512],
                                    start=(ct == 0), stop=False,
                                )
                    for i, (tt, psy) in enumerate(chains):
                        for co in range(2):
                            nc.tensor.matmul(
                                psy[:, co * 512:(co + 1) * 512],
                                attnT[:, 3, tt * P:(tt + 1) * P],
                                wp_sb[:, 3, co * 512:(co + 1) * 512],
                                start=False, stop=True,
                            )
                        yt = ystagep.tile([P, 1024], BF16, tag="ytf")
                        if i % 2 == 0:
                            nc.scalar.copy(yt[:], psy[:])
                        else:
                            nc.vector.tensor_copy(yt[:], psy[:])
                        nc.sync.dma_start(y_r[:, tt, :], yt[:])
    nc.compile()
    return nc


def _host_consts():
    import ml_dtypes
    i_idx = np.arange(P, dtype=np.float32)[:, None]
    j_idx = np.arange(P, dtype=np.float32)[None, :]
    maskb = np.where(j_idx - i_idx >= 0, 0.0, -1e30).astype(ml_dtypes.bfloat16)
    ident = np.eye(P, dtype=ml_dtypes.bfloat16)
    return ident, maskb


def make_in_maps(x, w_attn, b_attn, w_proj):
    import ml_dtypes
    bf16 = ml_dtypes.bfloat16
    ident, maskb = _host_consts()
    in_maps = []
    for c in range(8):
        b = c // 2
        h0 = (c % 2) * HPC
        qcols = slice(h0 * HD, h0 * HD + CL)
        kcols = slice(C + h0 * HD, C + h0 * HD + CL)
        vcols = slice(2 * C + h0 * HD, 2 * C + h0 * HD + CL)
        wqk = np.concatenate([w_attn[:, qcols], w_attn[:, kcols]], axis=1)
        bqk = np.concatenate([b_attn[qcols], b_attn[kcols]]).reshape(8, P).T
        in_maps.append({
            "xt": np.ascontiguousarray(x[b].T.astype(bf16)),
            "wqk": np.ascontiguousarray(wqk.astype(bf16)),
            "wv": np.ascontiguousarray(w_attn[:, vcols].astype(bf16)),
            "wp": np.ascontiguousarray(
                w_proj[h0 * HD:h0 * HD + CL, :].astype(bf16)),
            "bqk": np.ascontiguousarray(bqk),
            "ident": ident,
            "maskb": maskb,
        })
    return in_maps


def _get_runner():
    """Build the SPMD executor once: a cached jax.jit over 8 cores.

    Mirrors bass2jax.run_bass_via_pjrt but hoists the jit so repeated
    kernel() calls reuse the compiled executable.
    """
    if "runner" in _CACHED:
        return _CACHED["runner"]
    import jax
    import jax.numpy as jnp
    from jax.sharding import Mesh, PartitionSpec
    from jax.experimental.shard_map import shard_map
    from concourse import bass2jax
    import concourse.mybir as mybir_

    nc = _CACHED.get("nc")
    if nc is None:
        nc = _CACHED["nc"] = build_nc()
    bass2jax.install_neuronx_cc_hook()

    partition_name = (nc.partition_id_tensor.name
                      if nc.partition_id_tensor else None)
    in_names, out_names, out_avals, zero_shapes = [], [], [], []
    for alloc in nc.m.functions[0].allocations:
        if not isinstance(alloc, mybir_.MemoryLocationSet):
            continue
        name = alloc.memorylocations[0].name
        if alloc.kind == "ExternalInput":
            if name != partition_name:
                in_names.append(name)
        elif alloc.kind == "ExternalOutput":
            shape = tuple(alloc.tensor_shape)
            dtype = mybir_.dt.np(alloc.dtype)
            out_names.append(name)
            out_avals.append(jax.core.ShapedArray(shape, dtype))
            zero_shapes.append((shape, dtype))
    n_params = len(in_names)
    n_outs = len(out_names)
    all_names = in_names + out_names
    if partition_name is not None:
        all_names = all_names + [partition_name]

    def _body(*args):
        operands = list(args)
        if partition_name is not None:
            operands.append(bass2jax.partition_id_tensor())
        outs = bass2jax._bass_exec_p.bind(
            *operands,
            out_avals=tuple(out_avals),
            in_names=tuple(all_names),
            out_names=tuple(out_names),
            lowering_input_output_aliases=(),
            sim_require_finite=True,
            sim_require_nnan=True,
            nc=nc,
        )
        return tuple(outs)

    devices = jax.devices()[:8]
    mesh = Mesh(np.asarray(devices), ("core",))
    in_specs = (PartitionSpec("core"),) * (n_params + n_outs)
    out_specs = (PartitionSpec("core"),) * n_outs
    donate = tuple(range(n_params, n_params + n_outs))
    sharded = jax.jit(
        shard_map(_body, mesh=mesh, in_specs=in_specs, out_specs=out_specs,
                  check_rep=False),
        donate_argnums=donate, keep_unused=True,
    )

    def run(in_maps):
        concat_in = [
            np.concatenate([np.asarray(in_maps[c][nm]) for c in range(8)],
                           axis=0)
            for nm in in_names
        ]
        concat_zeros = [
            np.zeros((8 * s[0], *s[1:]), dt) for (s, dt) in zero_shapes
        ]
        out_arrs = sharded(*concat_in, *concat_zeros)
        return [
            {nm: np.asarray(out_arrs[i]).reshape(8, *out_avals[i].shape)[c]
             for i, nm in enumerate(out_names)}
            for c in range(8)
        ]

    _CACHED["runner"] = run
    return run


def kernel(x, w_attn, b_attn, w_proj, b_proj):
    x = np.asarray(x, dtype=np.float32)
    w_attn = np.asarray(w_attn, dtype=np.float32)
    b_attn = np.asarray(b_attn, dtype=np.float32)
    w_proj = np.asarray(w_proj, dtype=np.float32)
    b_proj = np.asarray(b_proj, dtype=np.float32)

    in_maps = make_in_maps(x, w_attn, b_attn, w_proj)
    try:
        run = _get_runner()
        results = run(in_maps)
    except Exception:
        # fallback: the stock SPMD runner (slower per call, same result)
        if "nc" not in _CACHED:
            _CACHED["nc"] = build_nc()
        res = run_bass_kernel_spmd(
            _CACHED["nc"], in_maps, core_ids=list(range(8)))
        results = res.results

    # v-bias contribution: probs rows sum to 1, so attn += 1 * b_v^T, and
    # (1 b_v^T) @ w_proj = row vector b_v @ w_proj added to every position.
    extra = b_attn[2 * C:] @ w_proj + b_proj  # [C]
    out = np.empty((B, T, C), dtype=np.float32)
    for b in range(B):
        out[b] = (results[2 * b]["y"].astype(np.float32)
                  + results[2 * b + 1]["y"].astype(np.float32) + extra)
    return out



# revision 22
# speedup vs baseline: 1.2262x; 1.0995x over previous
"""Causal multi-head attention (B=4, T=2048, C=1024, H=16) on 8 Trainium2 cores.

Sharding: core c handles batch b = c//2 and heads h0..h0+7 with h0 = (c%2)*8.
Each core computes QKV projection for its head slice, causal attention for its
8 heads, and a partial output projection. Host sums the two partials per batch
and adds the bias terms.

All matmuls run as float32r (full-speed fp32 on the PE, ~2e-4 relative error).

Attention works in the S^T = K Q^T layout ([k, q], k on partitions) so that
softmax normalization needs no cross-partition reduction: an extra all-ones
column appended to V makes the AV matmul emit the softmax row-sums for free,
and the causal mask is added into PSUM with an identity-matmul of a -1e30
bias tile before the exp.
"""

import os
import sys
import numpy as np

sys.path.insert(0, "/opt/trn_rl_repo")

import concourse.bass as bass  # noqa: E402
import concourse.bacc as bacc  # noqa: E402
import concourse.mybir as mybir  # noqa: E402
from concourse.bass_utils import run_bass_kernel_spmd  # noqa: E402
from concourse.tile import TileContext  # noqa: E402

B, T, C, H = 4, 2048, 1024, 16
HD = C // H          # 64 head dim
HPC = 8              # heads per core
P = 128
NT = T // P          # 16 t-chunks of 128
NS = T // 512        # 4 q-strips of 512
KC = C // P          # 8 contraction chunks for QKV
CL = HPC * HD        # 512 local channels per section
F32 = mybir.dt.float32
F32R = mybir.dt.float32r
BF16 = mybir.dt.bfloat16
F8 = mybir.dt.float8e4
DRM = mybir.MatmulPerfMode.DoubleRow
EXPF = mybir.ActivationFunctionType.Exp
MUL = mybir.AluOpType.mult
WS = 64.0            # host-side w_attn scale for fp8 dynamic range

_CACHED = {}


def build_nc():
    nc = bacc.Bacc("TRN2", target_bir_lowering=False, debug=False)

    xth_d = nc.dram_tensor("xth", [C, T], F8, kind="ExternalInput")
    xtl_d = nc.dram_tensor("xtl", [C, T], F8, kind="ExternalInput")
    wqkh_d = nc.dram_tensor("wqkh", [C, 2 * CL], F8, kind="ExternalInput")
    wqkl_d = nc.dram_tensor("wqkl", [C, 2 * CL], F8, kind="ExternalInput")
    wvh_d = nc.dram_tensor("wvh", [C, CL], F8, kind="ExternalInput")
    wvl_d = nc.dram_tensor("wvl", [C, CL], F8, kind="ExternalInput")
    wp_d = nc.dram_tensor("wp", [CL, C], BF16, kind="ExternalInput")
    bqk_d = nc.dram_tensor("bqk", [P, 8], F32, kind="ExternalInput")
    ident_d = nc.dram_tensor("ident", [P, P], BF16, kind="ExternalInput")
    maskb_d = nc.dram_tensor("maskb", [P, P], BF16, kind="ExternalInput")
    y_d = nc.dram_tensor("y", [T, C], BF16, kind="ExternalOutput")

    xth_r = xth_d.ap().rearrange("(kc p) t -> p kc t", p=P)     # [128, 8, 2048]
    xtl_r = xtl_d.ap().rearrange("(kc p) t -> p kc t", p=P)
    wqkh_r = wqkh_d.ap().rearrange("(kc p) c -> p kc c", p=P)   # [128, 8, 1024]
    wqkl_r = wqkl_d.ap().rearrange("(kc p) c -> p kc c", p=P)
    wvh_r = wvh_d.ap().rearrange("(kc p) c -> p kc c", p=P)     # [128, 8, 512]
    wvl_r = wvl_d.ap().rearrange("(kc p) c -> p kc c", p=P)
    wp_r = wp_d.ap().rearrange("(ct p) c -> p ct c", p=P)       # [128, 4, 1024]
    y_r = y_d.ap().rearrange("(tt p) c -> p tt c", p=P)         # [128, 16, 1024]

    SW = 512            # phase-A t-strip width
    EH = HD + 1         # 65: head slot width in v (value cols + ones col)
    scale = float(HD) ** -0.5 / (WS * WS)

    with TileContext(nc) as tc:
      with tc.tile_pool(name="const", bufs=1) as constp:
        with tc.tile_pool(name="qkv_big", bufs=1) as bigp:
            v_sb = bigp.tile([P, NT, HPC * EH], BF16)
            v_heads = v_sb[:].rearrange("p t (h e) -> p t h e", e=EH)
            qkT = bigp.tile([P, 8, T], BF16)  # c-tiles 0-3 = qT, 4-7 = kT

            # ---------------- Phase A: QKV projections ----------------
            # Single pass per 256-wide t-strip: V matmuls then qT/kT matmuls.
            # All DMAs are chunked per contraction block so the PE can start
            # as soon as the first chunks land; wqk chunks stream during the
            # early v work.
            with (
                tc.tile_pool(name="xts", bufs=3) as xtsp,
                tc.tile_pool(name="wqkv", bufs=1) as wqkvp,
                tc.tile_pool(name="ps_a", bufs=3, space="PSUM") as ps_a,
            ):
                wvh_sb = wqkvp.tile([P, KC, CL], F8)
                wvl_sb = wqkvp.tile([P, KC, CL], F8)
                wqkh_sb = wqkvp.tile([P, KC, 2 * CL], F8)
                wqkl_sb = wqkvp.tile([P, KC, 2 * CL], F8)
                xtsh0 = xtsp.tile([P, KC, SW], F8, tag="xh")
                xtsl0 = xtsp.tile([P, KC, SW], F8, tag="xl")
                # strip-0 feeds in half-chunks (kc 0-3 / 4-7) in the exact
                # order the DoubleRow products consume them
                for h in range(2):
                    hs = slice(4 * h, 4 * h + 4)
                    nc.sync.dma_start(xtsh0[:, hs, :], xth_r[:, hs, 0:SW])
                    nc.sync.dma_start(wvh_sb[:, hs, :], wvh_r[:, hs, :])
                    nc.sync.dma_start(xtsl0[:, hs, :], xtl_r[:, hs, 0:SW])
                    nc.sync.dma_start(wvl_sb[:, hs, :], wvl_r[:, hs, :])
                bqk = constp.tile([P, 8], F32)
                nc.sync.dma_start(bqk[:], bqk_d[:])
                # wqk streams on the Activation hwdge queue in parallel
                for h in range(2):
                    hs = slice(4 * h, 4 * h + 4)
                    nc.scalar.dma_start(wqkh_sb[:, hs, :], wqkh_r[:, hs, :])
                    nc.scalar.dma_start(wqkl_sb[:, hs, :], wqkl_r[:, hs, :])
                ident = constp.tile([P, P], BF16)
                maskb = constp.tile([P, P], BF16)

                NKP = KC // 2
                for ts in range(T // SW):
                    if ts == 0:
                        xtsh, xtsl = xtsh0, xtsl0
                    else:
                        xtsh = xtsp.tile([P, KC, SW], F8, tag="xh")
                        xtsl = xtsp.tile([P, KC, SW], F8, tag="xl")
                        nc.sync.dma_start(
                            xtsh[:], xth_r[:, :, ts * SW:(ts + 1) * SW])
                        nc.sync.dma_start(
                            xtsl[:], xtl_r[:, :, ts * SW:(ts + 1) * SW])
                    # v part: [t, c] orientation; x stationary, wv moving.
                    # 3-product compensated fp8 DoubleRow (hh, lh, hl).
                    for tt in range(SW // P):
                        tch = ts * (SW // P) + tt
                        tsl = slice(tt * P, (tt + 1) * P)
                        psv = ps_a.tile([P, CL], F32, tag="psa")
                        for kp in range(NKP):
                            s = slice(2 * kp, 2 * kp + 2)
                            nc.tensor.matmul(
                                psv[:], xtsh[:, s, tsl], wvh_sb[:, s, :],
                                start=(kp == 0), stop=False, perf_mode=DRM)
                            nc.tensor.matmul(
                                psv[:], xtsl[:, s, tsl], wvh_sb[:, s, :],
                                start=False, stop=False, perf_mode=DRM)
                            nc.tensor.matmul(
                                psv[:], xtsh[:, s, tsl], wvl_sb[:, s, :],
                                start=False, stop=(kp == NKP - 1),
                                perf_mode=DRM)
                        nc.vector.tensor_copy(
                            v_heads[:, tch, :, 0:HD],
                            psv[:].rearrange("p (h d) -> p h d", d=HD),
                        )
                    # qT/kT part: [c, t] orientation; w stationary, x moving
                    for ct in range(8):
                        csl = slice(ct * P, (ct + 1) * P)
                        psq = ps_a.tile([P, SW], F32, tag="psq")
                        for kp in range(NKP):
                            s = slice(2 * kp, 2 * kp + 2)
                            nc.tensor.matmul(
                                psq[:], wqkh_sb[:, s, csl], xtsh[:, s, :],
                                start=(kp == 0), stop=False, perf_mode=DRM)
                            nc.tensor.matmul(
                                psq[:], wqkl_sb[:, s, csl], xtsh[:, s, :],
                                start=False, stop=False, perf_mode=DRM)
                            nc.tensor.matmul(
                                psq[:], wqkh_sb[:, s, csl], xtsl[:, s, :],
                                start=False, stop=(kp == NKP - 1),
                                perf_mode=DRM)
                        nc.vector.tensor_scalar_add(
                            qkT[:, ct, ts * SW:(ts + 1) * SW],
                            psq[:],
                            bqk[:, ct:ct + 1],
                        )
                    if ts == 0:
                        # phase-B constants ride behind the early x strips;
                        # the ones column is a memset, not a DMA scatter
                        nc.sync.dma_start(ident[:], ident_d[:])
                        nc.sync.dma_start(maskb[:], maskb_d[:])
                        nc.vector.memset(v_heads[:, :, :, HD], 1.0)

            # ---------------- Phase B: attention + overlapped projection ----
            # Strip-major over q; heads run in pairs sharing a qkT c-tile
            # (rows 0-63 / 64-127 -> different PE row groups). The AV matmuls
            # lag the score matmuls by LAG k-tiles so the PE never waits on
            # the exp. Each pair is normalized right after its AV finishes
            # (fast-reciprocal + gpsimd row broadcast), and the previous
            # strip's output projection is interleaved into the current
            # strip's attention.
            with (
                tc.tile_pool(name="attnT_p", bufs=1) as attnTp,
                tc.tile_pool(name="proj", bufs=1) as projp,
                tc.tile_pool(name="ystage", bufs=4) as ystagep,
            ):
                attnT = attnTp.tile([P, 4, T], BF16)
                wp_sb = projp.tile([P, 4, C], BF16)

                with (
                    tc.tile_pool(name="u_pool", bufs=6) as up,
                    tc.tile_pool(name="attn_small", bufs=2) as smallp,
                    tc.tile_pool(name="attn_one", bufs=1) as small1p,
                    tc.tile_pool(name="o_un", bufs=5) as ounp,
                    tc.tile_pool(name="ps_s", bufs=2, space="PSUM") as ps_s,
                    tc.tile_pool(name="ps_o", bufs=2, space="PSUM") as ps_o,
                ):
                    LAG = 3

                    def proj_tiles(tt, ps_y):
                        for co in range(2):
                            psy = ps_y.tile([P, 512], F32, tag="psy")
                            for ct in range(4):
                                nc.tensor.matmul(
                                    psy[:],
                                    attnT[:, ct, tt * P:(tt + 1) * P],
                                    wp_sb[:, ct, co * 512:(co + 1) * 512],
                                    start=(ct == 0), stop=(ct == 3),
                                )
                            yt = ystagep.tile([P, 512], BF16, tag="yt")
                            nc.vector.tensor_copy(yt[:], psy[:])
                            nc.sync.dma_start(
                                y_r[:, tt, co * 512:(co + 1) * 512], yt[:])

                    nc.sync.dma_start(wp_sb[:], wp_r)
                    from contextlib import ExitStack
                    _stk = ExitStack()
                    ps_x = _stk.enter_context(
                        tc.tile_pool(name="ps_x", bufs=1, space="PSUM"))
                    ps_y = None
                    _nps = 0
                    for qj in range(NS):
                        if qj == 1:
                            _stk.close()  # release strip-0 extra psum
                            _stk = ExitStack()
                            ps_x = None
                            ps_y = _stk.enter_context(
                                tc.tile_pool(name="ps_y", bufs=2,
                                             space="PSUM"))
                        nk = 4 * (qj + 1)
                        for pr in range(4):  # head pair (2pr, 2pr+1)
                            qct, kct = pr, 4 + pr
                            psoA = ps_o.tile([EH, 512], F32, tag="ps_o")
                            psoB = ps_o.tile([EH, 512], F32, tag="ps_o")
                            u_ring = {}
                            for step in range(nk + LAG):
                                if step < nk:
                                    kt = step
                                    # columns < q0 of a diagonal tile are
                                    # fully masked: skip them in the scores,
                                    # exp, and AV; only the [128,128] block
                                    # at the diagonal needs the -1e30 mask.
                                    q0 = max(0, kt * P - qj * 512)
                                    diag = kt >= 4 * qj
                                    if ps_x is not None and _nps % 3 == 2:
                                        ps = ps_x.tile([P, 2, 512], F32,
                                                       tag="ps_x")
                                    else:
                                        ps = ps_s.tile([P, 2, 512], F32,
                                                       tag="ps_s")
                                    _nps += 1
                                    u = up.tile([P, 2, 512], BF16, tag="u")
                                    u_ring[kt] = u
                                    for hh in range(2):
                                        hp = hh * HD
                                        nc.tensor.matmul(
                                            ps[:, hh, q0:512],
                                            qkT[hp:hp + HD, kct,
                                                kt * P:(kt + 1) * P],
                                            qkT[hp:hp + HD, qct,
                                                qj * 512 + q0:
                                                (qj + 1) * 512],
                                            start=True, stop=not diag,
                                        )
                                    if diag:
                                        for hh in range(2):
                                            nc.tensor.matmul(
                                                ps[:, hh, q0:q0 + P],
                                                ident[:],
                                                maskb[:],
                                                start=False, stop=True,
                                            )
                                    nc.scalar.activation(
                                        u[:, :, q0:512], ps[:, :, q0:512],
                                        EXPF, scale=scale,
                                    )
                                if step >= LAG:
                                    kt = step - LAG
                                    u = u_ring.pop(kt)
                                    q0 = max(0, kt * P - qj * 512)
                                    last = kt == nk - 1
                                    nc.tensor.matmul(
                                        psoA[0:EH, q0:512],
                                        v_sb[:, kt,
                                             (2 * pr) * EH:(2 * pr + 1) * EH],
                                        u[:, 0, q0:512],
                                        start=(kt == 0), stop=last,
                                    )
                                    nc.tensor.matmul(
                                        psoB[0:EH, q0:512],
                                        v_sb[:, kt,
                                             (2 * pr + 1) * EH:
                                             (2 * pr + 2) * EH],
                                        u[:, 1, q0:512],
                                        start=(kt == 0), stop=last,
                                    )
                            # per-pair normalize: in-place reciprocal on
                            # the rowsum row (partition 64) + direct gpsimd
                            # broadcast from it -- no DMAs except the odd-head
                            # cross-partition store into attnT rows 64-127.
                            qsl = slice(qj * 512, (qj + 1) * 512)
                            # previous strip's projection: issued first so its
                            # psum-freeing copies sit ahead of the normalize
                            # chain in the DVE queue
                            if qj > 0:
                                proj_tiles(4 * (qj - 1) + pr, ps_y)
                            ounA = ounp.tile([EH, 512], F32, tag="oun")
                            nc.vector.tensor_copy(ounA[:], psoA[:])
                            ounB = ounp.tile([EH, 512], F32, tag="oun")
                            nc.vector.tensor_copy(ounB[:], psoB[:])
                            # cross-partition reciprocal p64 -> p0 (DVE allows
                            # partition remaps between aligned bases); the
                            # broadcast source must sit at partition 0 on hw
                            rcA = smallp.tile([1, 512], F32, tag="rc")
                            nc.vector.reciprocal(rcA[:], ounA[HD:EH, :])
                            rcB = smallp.tile([1, 512], F32, tag="rc")
                            nc.vector.reciprocal(rcB[:], ounB[HD:EH, :])
                            bcA = smallp.tile([HD, 512], F32, tag="bc")
                            nc.gpsimd.partition_broadcast(bcA[:], rcA[:])
                            bcB = smallp.tile([HD, 512], F32, tag="bc")
                            nc.gpsimd.partition_broadcast(bcB[:], rcB[:])
                            nc.vector.tensor_tensor(
                                attnT[0:HD, pr, qsl],
                                ounA[0:HD, :], bcA[:], MUL,
                            )
                            # odd head: DVE cross-partition write (0-63 ->
                            # 64-127), aligned partition bases
                            nc.vector.tensor_tensor(
                                attnT[HD:P, pr, qsl],
                                ounB[0:HD, :], bcB[:], MUL,
                            )
                    _stk.close()
                # last strip's projection with a wide psum pool (the
                # attention pools are closed by now) so its first three
                # accumulation steps overlap the final normalize chain
                with tc.tile_pool(name="ps_f", bufs=4, space="PSUM") as ps_f:
                    chains = []
                    for tt in range(4 * (NS - 1), 4 * NS):
                        chains.append(
                            (tt, ps_f.tile([P, 1024], F32, tag="psf",
                                           name=f"psf_{tt}")))
                    # stage-major so every chain's first three accumulation
                    # steps can run while the last head pair normalizes; the
                    # last stage goes chain-major so each chain's copy + DMA
                    # launches as soon as its accumulation closes
                    for ct in range(3):
                        for tt, psy in chains:
                            for co in range(2):
                                nc.tensor.matmul(
                                    psy[:, co * 512:(co + 1) * 512],
                                    attnT[:, ct, tt * P:(tt + 1) * P],
                                    wp_sb[:, ct, co * 512:(co + 1) * 512],
                                    start=(ct == 0), stop=False,
                                )
                    for i, (tt, psy) in enumerate(chains):
                        for co in range(2):
                            nc.tensor.matmul(
                                psy[:, co * 512:(co + 1) * 512],
                                attnT[:, 3, tt * P:(tt + 1) * P],
                                wp_sb[:, 3, co * 512:(co + 1) * 512],
                                start=False, stop=True,
                            )
                        yt = ystagep.tile([P, 1024], BF16, tag="ytf")
                        if i % 2 == 0:
                            nc.scalar.copy(yt[:], psy[:])
                        else:
                            nc.vector.tensor_copy(yt[:], psy[:])
                        nc.sync.dma_start(y_r[:, tt, :], yt[:])
    nc.compile()
    return nc


def _host_consts():
    import ml_dtypes
    i_idx = np.arange(P, dtype=np.float32)[:, None]
    j_idx = np.arange(P, dtype=np.float32)[None, :]
    maskb = np.where(j_idx - i_idx >= 0, 0.0, -1e30).astype(ml_dtypes.bfloat16)
    ident = np.eye(P, dtype=ml_dtypes.bfloat16)
    return ident, maskb


def make_in_maps(x, w_attn, b_attn, w_proj):
    import ml_dtypes
    bf16 = ml_dtypes.bfloat16
    f8 = ml_dtypes.float8_e4m3
    ident, maskb = _host_consts()
    in_maps = []
    for c in range(8):
        b = c // 2
        h0 = (c % 2) * HPC
        qcols = slice(h0 * HD, h0 * HD + CL)
        kcols = slice(C + h0 * HD, C + h0 * HD + CL)
        vcols = slice(2 * C + h0 * HD, 2 * C + h0 * HD + CL)
        wqk = np.concatenate([w_attn[:, qcols], w_attn[:, kcols]],
                             axis=1) * np.float32(WS)
        wv = w_attn[:, vcols] * np.float32(WS)
        bqk = (np.concatenate([b_attn[qcols], b_attn[kcols]])
               * np.float32(WS)).reshape(8, P).T
        xt = np.ascontiguousarray(x[b].T)
        xth = xt.astype(f8)
        xtl = (xt - xth.astype(np.float32)).astype(f8)
        wqkh = wqk.astype(f8)
        wqkl = (wqk - wqkh.astype(np.float32)).astype(f8)
        wvh = wv.astype(f8)
        wvl = (wv - wvh.astype(np.float32)).astype(f8)
        in_maps.append({
            "xth": xth, "xtl": np.ascontiguousarray(xtl),
            "wqkh": np.ascontiguousarray(wqkh),
            "wqkl": np.ascontiguousarray(wqkl),
            "wvh": np.ascontiguousarray(wvh),
            "wvl": np.ascontiguousarray(wvl),
            "wp": np.ascontiguousarray(
                (w_proj[h0 * HD:h0 * HD + CL, :]
                 / np.float32(WS)).astype(bf16)),
            "bqk": np.ascontiguousarray(bqk),
            "ident": ident,
            "maskb": maskb,
        })
    return in_maps


def _get_runner():
    """Build the SPMD executor once: a cached jax.jit over 8 cores.

    Mirrors bass2jax.run_bass_via_pjrt but hoists the jit so repeated
    kernel() calls reuse the compiled executable.
    """
    if "runner" in _CACHED:
        return _CACHED["runner"]
    import jax
    import jax.numpy as jnp
    from jax.sharding import Mesh, PartitionSpec
    from jax.experimental.shard_map import shard_map
    from concourse import bass2jax
    import concourse.mybir as mybir_

    nc = _CACHED.get("nc")
    if nc is None:
        nc = _CACHED["nc"] = build_nc()
    bass2jax.install_neuronx_cc_hook()

    partition_name = (nc.partition_id_tensor.name
                      if nc.partition_id_tensor else None)
    in_names, out_names, out_avals, zero_shapes = [], [], [], []
    for alloc in nc.m.functions[0].allocations:
        if not isinstance(alloc, mybir_.MemoryLocationSet):
            continue
        name = alloc.memorylocations[0].name
        if alloc.kind == "ExternalInput":
            if name != partition_name:
                in_names.append(name)
        elif alloc.kind == "ExternalOutput":
            shape = tuple(alloc.tensor_shape)
            dtype = mybir_.dt.np(alloc.dtype)
            out_names.append(name)
            out_avals.append(jax.core.ShapedArray(shape, dtype))
            zero_shapes.append((shape, dtype))
    n_params = len(in_names)
    n_outs = len(out_names)
    all_names = in_names + out_names
    if partition_name is not None:
        all_names = all_names + [partition_name]

    def _body(*args):
        operands = list(args)
        if partition_name is not None:
            operands.append(bass2jax.partition_id_tensor())
        outs = bass2jax._bass_exec_p.bind(
            *operands,
            out_avals=tuple(out_avals),
            in_names=tuple(all_names),
            out_names=tuple(out_names),
            lowering_input_output_aliases=(),
            sim_require_finite=True,
            sim_require_nnan=True,
            nc=nc,
        )
        return tuple(outs)

    devices = jax.devices()[:8]
    mesh = Mesh(np.asarray(devices), ("core",))
    in_specs = (PartitionSpec("core"),) * (n_params + n_outs)
    out_specs = (PartitionSpec("core"),) * n_outs
    donate = tuple(range(n_params, n_params + n_outs))
    sharded = jax.jit(
        shard_map(_body, mesh=mesh, in_specs=in_specs, out_specs=out_specs,
                  check_rep=False),
        donate_argnums=donate, keep_unused=True,
    )

    def run(in_maps):
        concat_in = [
            np.concatenate([np.asarray(in_maps[c][nm]) for c in range(8)],
                           axis=0)
            for nm in in_names
        ]
        concat_zeros = [
            np.zeros((8 * s[0], *s[1:]), dt) for (s, dt) in zero_shapes
        ]
        out_arrs = sharded(*concat_in, *concat_zeros)
        return [
            {nm: np.asarray(out_arrs[i]).reshape(8, *out_avals[i].shape)[c]
             for i, nm in enumerate(out_names)}
            for c in range(8)
        ]

    _CACHED["runner"] = run
    return run


def kernel(x, w_attn, b_attn, w_proj, b_proj):
    x = np.asarray(x, dtype=np.float32)
    w_attn = np.asarray(w_attn, dtype=np.float32)
    b_attn = np.asarray(b_attn, dtype=np.float32)
    w_proj = np.asarray(w_proj, dtype=np.float32)
    b_proj = np.asarray(b_proj, dtype=np.float32)

    in_maps = make_in_maps(x, w_attn, b_attn, w_proj)
    try:
        run = _get_runner()
        results = run(in_maps)
    except Exception:
        # fallback: the stock SPMD runner (slower per call, same result)
        if "nc" not in _CACHED:
            _CACHED["nc"] = build_nc()
        res = run_bass_kernel_spmd(
            _CACHED["nc"], in_maps, core_ids=list(range(8)))
        results = res.results

    # v-bias contribution: probs rows sum to 1, so attn += 1 * b_v^T, and
    # (1 b_v^T) @ w_proj = row vector b_v @ w_proj added to every position.
    extra = b_attn[2 * C:] @ w_proj + b_proj  # [C]
    out = np.empty((B, T, C), dtype=np.float32)
    for b in range(B):
        out[b] = (results[2 * b]["y"].astype(np.float32)
                  + results[2 * b + 1]["y"].astype(np.float32) + extra)
    return out



# revision 31
# speedup vs baseline: 1.2362x; 1.0081x over previous
"""Causal multi-head attention (B=4, T=2048, C=1024, H=16) on 8 Trainium2 cores.

Sharding: core c handles batch b = c//2 and heads h0..h0+7 with h0 = (c%2)*8.
Each core computes QKV projection for its head slice, causal attention for its
8 heads, and a partial output projection. Host sums the two partials per batch
and adds the bias terms.

All matmuls run as float32r (full-speed fp32 on the PE, ~2e-4 relative error).

Attention works in the S^T = K Q^T layout ([k, q], k on partitions) so that
softmax normalization needs no cross-partition reduction: an extra all-ones
column appended to V makes the AV matmul emit the softmax row-sums for free,
and the causal mask is added into PSUM with an identity-matmul of a -1e30
bias tile before the exp.
"""

import os
import sys
import numpy as np

sys.path.insert(0, "/opt/trn_rl_repo")

import concourse.bass as bass  # noqa: E402
import concourse.bacc as bacc  # noqa: E402
import concourse.mybir as mybir  # noqa: E402
from concourse.bass_utils import run_bass_kernel_spmd  # noqa: E402
from concourse.tile import TileContext  # noqa: E402

B, T, C, H = 4, 2048, 1024, 16
HD = C // H          # 64 head dim
HPC = 8              # heads per core
P = 128
NT = T // P          # 16 t-chunks of 128
NS = T // 512        # 4 q-strips of 512
KC = C // P          # 8 contraction chunks for QKV
CL = HPC * HD        # 512 local channels per section
F32 = mybir.dt.float32
F32R = mybir.dt.float32r
BF16 = mybir.dt.bfloat16
F8 = mybir.dt.float8e4
DRM = mybir.MatmulPerfMode.DoubleRow
EXPF = mybir.ActivationFunctionType.Exp
MUL = mybir.AluOpType.mult
WS = 64.0            # host-side w_attn scale for fp8 dynamic range

_CACHED = {}


def build_nc():
    nc = bacc.Bacc("TRN2", target_bir_lowering=False, debug=False)

    xth_d = nc.dram_tensor("xth", [C, T], F8, kind="ExternalInput")
    xtl_d = nc.dram_tensor("xtl", [C, T], F8, kind="ExternalInput")
    wqkh_d = nc.dram_tensor("wqkh", [C, 2 * CL], F8, kind="ExternalInput")
    wqkl_d = nc.dram_tensor("wqkl", [C, 2 * CL], F8, kind="ExternalInput")
    wvh_d = nc.dram_tensor("wvh", [C, CL], F8, kind="ExternalInput")
    wvl_d = nc.dram_tensor("wvl", [C, CL], F8, kind="ExternalInput")
    wp_d = nc.dram_tensor("wp", [CL, C], BF16, kind="ExternalInput")
    ident_d = nc.dram_tensor("ident", [P, P], BF16, kind="ExternalInput")
    bqk_d = nc.dram_tensor("bqk", [P, 8], F32, kind="ExternalInput")
    maskb_d = nc.dram_tensor("maskb", [P, P], BF16, kind="ExternalInput")
    y_d = nc.dram_tensor("y", [T, C], BF16, kind="ExternalOutput")

    xth_r = xth_d.ap().rearrange("(kc p) t -> p kc t", p=P)     # [128, 8, 2048]
    xtl_r = xtl_d.ap().rearrange("(kc p) t -> p kc t", p=P)
    wqkh_r = wqkh_d.ap().rearrange("(kc p) c -> p kc c", p=P)   # [128, 8, 1024]
    wqkl_r = wqkl_d.ap().rearrange("(kc p) c -> p kc c", p=P)
    wvh_r = wvh_d.ap().rearrange("(kc p) c -> p kc c", p=P)     # [128, 8, 512]
    wvl_r = wvl_d.ap().rearrange("(kc p) c -> p kc c", p=P)
    wp_r = wp_d.ap().rearrange("(ct p) c -> p ct c", p=P)       # [128, 4, 1024]
    y_r = y_d.ap().rearrange("(tt p) c -> p tt c", p=P)         # [128, 16, 1024]

    SW = 512            # phase-A t-strip width
    EH = HD + 1         # 65: head slot width in v (value cols + ones col)
    scale = float(HD) ** -0.5 / (WS * WS)

    with TileContext(nc) as tc:
      with tc.tile_pool(name="const", bufs=1) as constp:
        with tc.tile_pool(name="qkv_big", bufs=1) as bigp:
            v_sb = bigp.tile([P, NT, HPC * EH], BF16)
            v_heads = v_sb[:].rearrange("p t (h e) -> p t h e", e=EH)
            qkT = bigp.tile([P, 8, T], BF16)  # c-tiles 0-3 = qT, 4-7 = kT

            # ---------------- Phase A: QKV projections ----------------
            # Single pass per 256-wide t-strip: V matmuls then qT/kT matmuls.
            # All DMAs are chunked per contraction block so the PE can start
            # as soon as the first chunks land; wqk chunks stream during the
            # early v work.
            with (
                tc.tile_pool(name="xts", bufs=3) as xtsp,
                tc.tile_pool(name="wqkv", bufs=1) as wqkvp,
                tc.tile_pool(name="ps_a", bufs=3, space="PSUM") as ps_a,
            ):
                wvh_sb = wqkvp.tile([P, KC, CL], F8)
                wvl_sb = wqkvp.tile([P, KC, CL], F8)
                wqkh_sb = wqkvp.tile([P, KC, 2 * CL], F8)
                wqkl_sb = wqkvp.tile([P, KC, 2 * CL], F8)
                xtsh0 = xtsp.tile([P, KC, SW], F8, tag="xh")
                xtsl0 = xtsp.tile([P, KC, SW], F8, tag="xl")
                # strip-0 feeds in half-chunks (kc 0-3 / 4-7) in the exact
                # order the DoubleRow products consume them
                # single ordered queue, sequenced by first-use time:
                # v-part hi feeds, v-part lo, then wqk hi/lo for the qk part
                for hs in (slice(0, 2), slice(2, 8)):
                    nc.sync.dma_start(xtsh0[:, hs, :], xth_r[:, hs, 0:SW])
                    nc.sync.dma_start(wvh_sb[:, hs, :], wvh_r[:, hs, :])
                nc.sync.dma_start(xtsl0[:], xtl_r[:, :, 0:SW])
                nc.sync.dma_start(wvl_sb[:], wvl_r[:])
                bqk = constp.tile([P, 8], F32)
                nc.sync.dma_start(bqk[:], bqk_d[:])
                for h in range(2):
                    hs = slice(4 * h, 4 * h + 4)
                    nc.sync.dma_start(wqkh_sb[:, hs, :], wqkh_r[:, hs, :])
                for h in range(2):
                    hs = slice(4 * h, 4 * h + 4)
                    nc.sync.dma_start(wqkl_sb[:, hs, :], wqkl_r[:, hs, :])
                ident = constp.tile([P, P], BF16)
                maskb = constp.tile([P, P], BF16)

                NKP = KC // 2
                for ts in range(T // SW):
                    if ts == 0:
                        xtsh, xtsl = xtsh0, xtsl0
                    else:
                        xtsh = xtsp.tile([P, KC, SW], F8, tag="xh")
                        xtsl = xtsp.tile([P, KC, SW], F8, tag="xl")
                        nc.sync.dma_start(
                            xtsh[:], xth_r[:, :, ts * SW:(ts + 1) * SW])
                        nc.sync.dma_start(
                            xtsl[:], xtl_r[:, :, ts * SW:(ts + 1) * SW])
                    # v part: [t, c] orientation; x stationary, wv moving.
                    # 3-product compensated fp8 DoubleRow (hh, lh, hl).
                    for tt in range(SW // P):
                        tch = ts * (SW // P) + tt
                        tsl = slice(tt * P, (tt + 1) * P)
                        psv = ps_a.tile([P, CL], F32, tag="psa")
                        for xt_, wv_, first, last in (
                            (xtsh, wvh_sb, True, False),
                            (xtsl, wvh_sb, False, False),
                            (xtsh, wvl_sb, False, True),
                        ):
                            for kp in range(NKP):
                                s = slice(2 * kp, 2 * kp + 2)
                                nc.tensor.matmul(
                                    psv[:], xt_[:, s, tsl], wv_[:, s, :],
                                    start=(first and kp == 0),
                                    stop=(last and kp == NKP - 1),
                                    perf_mode=DRM)
                        nc.vector.tensor_copy(
                            v_heads[:, tch, :, 0:HD],
                            psv[:].rearrange("p (h d) -> p h d", d=HD),
                        )
                    # qT/kT part: [c, t] orientation; w stationary, x moving
                    for ct in range(8):
                        csl = slice(ct * P, (ct + 1) * P)
                        psq = ps_a.tile([P, SW], F32, tag="psq")
                        for wq_, xt_, first, last in (
                            (wqkh_sb, xtsh, True, False),
                            (wqkl_sb, xtsh, False, False),
                            (wqkh_sb, xtsl, False, True),
                        ):
                            for kp in range(NKP):
                                s = slice(2 * kp, 2 * kp + 2)
                                nc.tensor.matmul(
                                    psq[:], wq_[:, s, csl], xt_[:, s, :],
                                    start=(first and kp == 0),
                                    stop=(last and kp == NKP - 1),
                                    perf_mode=DRM)
                        nc.vector.tensor_scalar_add(
                            qkT[:, ct, ts * SW:(ts + 1) * SW],
                            psq[:],
                            bqk[:, ct:ct + 1],
                        )
                    if ts == 0:
                        # phase-B constants ride behind the early x strips;
                        # the ones column is a memset, not a DMA scatter
                        nc.sync.dma_start(ident[:], ident_d[:])
                        nc.sync.dma_start(maskb[:], maskb_d[:])
                        nc.vector.memset(v_heads[:, :, :, HD], 1.0)

            # ---------------- Phase B: attention + overlapped projection ----
            # Strip-major over q; heads run in pairs sharing a qkT c-tile
            # (rows 0-63 / 64-127 -> different PE row groups). The AV matmuls
            # lag the score matmuls by LAG k-tiles so the PE never waits on
            # the exp. Each pair is normalized right after its AV finishes
            # (fast-reciprocal + gpsimd row broadcast), and the previous
            # strip's output projection is interleaved into the current
            # strip's attention.
            with (
                tc.tile_pool(name="attnT_p", bufs=1) as attnTp,
                tc.tile_pool(name="proj", bufs=1) as projp,
                tc.tile_pool(name="ystage", bufs=4) as ystagep,
            ):
                attnT = attnTp.tile([P, 4, T], BF16)
                wp_sb = projp.tile([P, 4, C], BF16)

                with (
                    tc.tile_pool(name="u_pool", bufs=6) as up,
                    tc.tile_pool(name="attn_small", bufs=2) as smallp,
                    tc.tile_pool(name="attn_one", bufs=1) as small1p,
                    tc.tile_pool(name="o_un", bufs=5) as ounp,
                    tc.tile_pool(name="ps_s", bufs=2, space="PSUM") as ps_s,
                    tc.tile_pool(name="ps_o", bufs=2, space="PSUM") as ps_o,
                ):
                    LAG = 3

                    def proj_tiles(tt, ps_y):
                        for co in range(2):
                            psy = ps_y.tile([P, 512], F32, tag="psy")
                            for ct in range(4):
                                nc.tensor.matmul(
                                    psy[:],
                                    attnT[:, ct, tt * P:(tt + 1) * P],
                                    wp_sb[:, ct, co * 512:(co + 1) * 512],
                                    start=(ct == 0), stop=(ct == 3),
                                )
                            yt = ystagep.tile([P, 512], BF16, tag="yt")
                            nc.vector.tensor_copy(yt[:], psy[:])
                            nc.sync.dma_start(
                                y_r[:, tt, co * 512:(co + 1) * 512], yt[:])

                    nc.sync.dma_start(wp_sb[:], wp_r)
                    from contextlib import ExitStack
                    _stk = ExitStack()
                    ps_x = _stk.enter_context(
                        tc.tile_pool(name="ps_x", bufs=1, space="PSUM"))
                    ps_y = None
                    _nps = 0
                    for qj in range(NS):
                        if qj == 1:
                            _stk.close()  # release strip-0 extra psum
                            _stk = ExitStack()
                            ps_x = None
                            ps_y = _stk.enter_context(
                                tc.tile_pool(name="ps_y", bufs=2,
                                             space="PSUM"))
                        nk = 4 * (qj + 1)
                        for pr in range(4):  # head pair (2pr, 2pr+1)
                            qct, kct = pr, 4 + pr
                            psoA = ps_o.tile([EH, 512], F32, tag="ps_o")
                            psoB = ps_o.tile([EH, 512], F32, tag="ps_o")
                            u_ring = {}
                            for step in range(nk + LAG):
                                if step < nk:
                                    kt = step
                                    # columns < q0 of a diagonal tile are
                                    # fully masked: skip them in the scores,
                                    # exp, and AV; only the [128,128] block
                                    # at the diagonal needs the -1e30 mask.
                                    q0 = max(0, kt * P - qj * 512)
                                    diag = kt >= 4 * qj
                                    if ps_x is not None and _nps % 3 == 2:
                                        ps = ps_x.tile([P, 2, 512], F32,
                                                       tag="ps_x")
                                    else:
                                        ps = ps_s.tile([P, 2, 512], F32,
                                                       tag="ps_s")
                                    _nps += 1
                                    u = up.tile([P, 2, 512], BF16, tag="u")
                                    u_ring[kt] = u
                                    for hh in range(2):
                                        hp = hh * HD
                                        nc.tensor.matmul(
                                            ps[:, hh, q0:512],
                                            qkT[hp:hp + HD, kct,
                                                kt * P:(kt + 1) * P],
                                            qkT[hp:hp + HD, qct,
                                                qj * 512 + q0:
                                                (qj + 1) * 512],
                                            start=True, stop=not diag,
                                        )
                                    if diag:
                                        for hh in range(2):
                                            nc.tensor.matmul(
                                                ps[:, hh, q0:q0 + P],
                                                ident[:],
                                                maskb[:],
                                                start=False, stop=True,
                                            )
                                    nc.scalar.activation(
                                        u[:, :, q0:512], ps[:, :, q0:512],
                                        EXPF, scale=scale,
                                    )
                                if step >= LAG:
                                    kt = step - LAG
                                    u = u_ring.pop(kt)
                                    q0 = max(0, kt * P - qj * 512)
                                    last = kt == nk - 1
                                    nc.tensor.matmul(
                                        psoA[0:EH, q0:512],
                                        v_sb[:, kt,
                                             (2 * pr) * EH:(2 * pr + 1) * EH],
                                        u[:, 0, q0:512],
                                        start=(kt == 0), stop=last,
                                    )
                                    nc.tensor.matmul(
                                        psoB[0:EH, q0:512],
                                        v_sb[:, kt,
                                             (2 * pr + 1) * EH:
                                             (2 * pr + 2) * EH],
                                        u[:, 1, q0:512],
                                        start=(kt == 0), stop=last,
                                    )
                            # per-pair normalize: in-place reciprocal on
                            # the rowsum row (partition 64) + direct gpsimd
                            # broadcast from it -- no DMAs except the odd-head
                            # cross-partition store into attnT rows 64-127.
                            qsl = slice(qj * 512, (qj + 1) * 512)
                            # previous strip's projection: issued first so its
                            # psum-freeing copies sit ahead of the normalize
                            # chain in the DVE queue
                            if qj > 0:
                                proj_tiles(4 * (qj - 1) + pr, ps_y)
                            ounA = ounp.tile([EH, 512], F32, tag="oun")
                            nc.vector.tensor_copy(ounA[:], psoA[:])
                            ounB = ounp.tile([EH, 512], F32, tag="oun")
                            nc.vector.tensor_copy(ounB[:], psoB[:])
                            # cross-partition reciprocal p64 -> p0 (DVE allows
                            # partition remaps between aligned bases); the
                            # broadcast source must sit at partition 0 on hw
                            rcA = smallp.tile([1, 512], F32, tag="rc")
                            nc.vector.reciprocal(rcA[:], ounA[HD:EH, :])
                            rcB = smallp.tile([1, 512], F32, tag="rc")
                            nc.vector.reciprocal(rcB[:], ounB[HD:EH, :])
                            bcA = smallp.tile([HD, 512], F32, tag="bc")
                            nc.gpsimd.partition_broadcast(bcA[:], rcA[:])
                            bcB = smallp.tile([HD, 512], F32, tag="bc")
                            nc.gpsimd.partition_broadcast(bcB[:], rcB[:])
                            nc.vector.tensor_tensor(
                                attnT[0:HD, pr, qsl],
                                ounA[0:HD, :], bcA[:], MUL,
                            )
                            # odd head: DVE cross-partition write (0-63 ->
                            # 64-127), aligned partition bases
                            nc.vector.tensor_tensor(
                                attnT[HD:P, pr, qsl],
                                ounB[0:HD, :], bcB[:], MUL,
                            )
                    _stk.close()
                # last strip's projection with a wide psum pool (the
                # attention pools are closed by now) so its first three
                # accumulation steps overlap the final normalize chain
                with tc.tile_pool(name="ps_f", bufs=4, space="PSUM") as ps_f:
                    chains = []
                    for tt in range(4 * (NS - 1), 4 * NS):
                        chains.append(
                            (tt, ps_f.tile([P, 1024], F32, tag="psf",
                                           name=f"psf_{tt}")))
                    # stage-major so every chain's first three accumulation
                    # steps can run while the last head pair normalizes; the
                    # last stage goes chain-major so each chain's copy + DMA
                    # launches as soon as its accumulation closes
                    for ct in range(3):
                        for tt, psy in chains:
                            for co in range(2):
                                nc.tensor.matmul(
                                    psy[:, co * 512:(co + 1) * 512],
                                    attnT[:, ct, tt * P:(tt + 1) * P],
                                    wp_sb[:, ct, co * 512:(co + 1) * 512],
                                    start=(ct == 0), stop=False,
                                )
                    for i, (tt, psy) in enumerate(chains):
                        for co in range(2):
                            nc.tensor.matmul(
                                psy[:, co * 512:(co + 1) * 512],
                                attnT[:, 3, tt * P:(tt + 1) * P],
                                wp_sb[:, 3, co * 512:(co + 1) * 512],
                                start=False, stop=True,
                            )
                        yt = ystagep.tile([P, 1024], BF16, tag="ytf")
                        if i % 2 == 0:
                            nc.scalar.copy(yt[:], psy[:])
                        else:
                            nc.vector.tensor_copy(yt[:], psy[:])
                        nc.sync.dma_start(y_r[:, tt, :], yt[:])
    nc.compile()
    return nc


def _host_consts():
    import ml_dtypes
    i_idx = np.arange(P, dtype=np.float32)[:, None]
    j_idx = np.arange(P, dtype=np.float32)[None, :]
    maskb = np.where(j_idx - i_idx >= 0, 0.0, -1e30).astype(ml_dtypes.bfloat16)
    ident = np.eye(P, dtype=ml_dtypes.bfloat16)
    return ident, maskb


def make_in_maps(x, w_attn, b_attn, w_proj):
    import ml_dtypes
    bf16 = ml_dtypes.bfloat16
    f8 = ml_dtypes.float8_e4m3
    ident, maskb = _host_consts()
    in_maps = []
    for c in range(8):
        b = c // 2
        h0 = (c % 2) * HPC
        qcols = slice(h0 * HD, h0 * HD + CL)
        kcols = slice(C + h0 * HD, C + h0 * HD + CL)
        vcols = slice(2 * C + h0 * HD, 2 * C + h0 * HD + CL)
        wqk = np.concatenate([w_attn[:, qcols], w_attn[:, kcols]],
                             axis=1) * np.float32(WS)
        wv = w_attn[:, vcols] * np.float32(WS)
        bqk = (np.concatenate([b_attn[qcols], b_attn[kcols]])
               * np.float32(WS)).reshape(8, P).T
        xt = np.ascontiguousarray(x[b].T)
        xth = xt.astype(f8)
        xtl = (xt - xth.astype(np.float32)).astype(f8)
        wqkh = wqk.astype(f8)
        wqkl = (wqk - wqkh.astype(np.float32)).astype(f8)
        wvh = wv.astype(f8)
        wvl = (wv - wvh.astype(np.float32)).astype(f8)
        in_maps.append({
            "xth": xth, "xtl": np.ascontiguousarray(xtl),
            "wqkh": np.ascontiguousarray(wqkh),
            "wqkl": np.ascontiguousarray(wqkl),
            "wvh": np.ascontiguousarray(wvh),
            "wvl": np.ascontiguousarray(wvl),
            "wp": np.ascontiguousarray(
                (w_proj[h0 * HD:h0 * HD + CL, :]
                 / np.float32(WS)).astype(bf16)),
            "bqk": np.ascontiguousarray(bqk),
            "ident": ident,
            "maskb": maskb,
        })
    return in_maps


def _get_runner():
    """Build the SPMD executor once: a cached jax.jit over 8 cores.

    Mirrors bass2jax.run_bass_via_pjrt but hoists the jit so repeated
    kernel() calls reuse the compiled executable.
    """
    if "runner" in _CACHED:
        return _CACHED["runner"]
    import jax
    import jax.numpy as jnp
    from jax.sharding import Mesh, PartitionSpec
    from jax.experimental.shard_map import shard_map
    from concourse import bass2jax
    import concourse.mybir as mybir_

    nc = _CACHED.get("nc")
    if nc is None:
        nc = _CACHED["nc"] = build_nc()
    bass2jax.install_neuronx_cc_hook()

    partition_name = (nc.partition_id_tensor.name
                      if nc.partition_id_tensor else None)
    in_names, out_names, out_avals, zero_shapes = [], [], [], []
    for alloc in nc.m.functions[0].allocations:
        if not isinstance(alloc, mybir_.MemoryLocationSet):
            continue
        name = alloc.memorylocations[0].name
        if alloc.kind == "ExternalInput":
            if name != partition_name:
                in_names.append(name)
        elif alloc.kind == "ExternalOutput":
            shape = tuple(alloc.tensor_shape)
            dtype = mybir_.dt.np(alloc.dtype)
            out_names.append(name)
            out_avals.append(jax.core.ShapedArray(shape, dtype))
            zero_shapes.append((shape, dtype))
    n_params = len(in_names)
    n_outs = len(out_names)
    all_names = in_names + out_names
    if partition_name is not None:
        all_names = all_names + [partition_name]

    def _body(*args):
        operands = list(args)
        if partition_name is not None:
            operands.append(bass2jax.partition_id_tensor())
        outs = bass2jax._bass_exec_p.bind(
            *operands,
            out_avals=tuple(out_avals),
            in_names=tuple(all_names),
            out_names=tuple(out_names),
            lowering_input_output_aliases=(),
            sim_require_finite=True,
            sim_require_nnan=True,
            nc=nc,
        )
        return tuple(outs)

    devices = jax.devices()[:8]
    mesh = Mesh(np.asarray(devices), ("core",))
    in_specs = (PartitionSpec("core"),) * (n_params + n_outs)
    out_specs = (PartitionSpec("core"),) * n_outs
    donate = tuple(range(n_params, n_params + n_outs))
    sharded = jax.jit(
        shard_map(_body, mesh=mesh, in_specs=in_specs, out_specs=out_specs,
                  check_rep=False),
        donate_argnums=donate, keep_unused=True,
    )

    def run(in_maps):
        concat_in = [
            np.concatenate([np.asarray(in_maps[c][nm]) for c in range(8)],
                           axis=0)
            for nm in in_names
        ]
        concat_zeros = [
            np.zeros((8 * s[0], *s[1:]), dt) for (s, dt) in zero_shapes
        ]
        out_arrs = sharded(*concat_in, *concat_zeros)
        return [
            {nm: np.asarray(out_arrs[i]).reshape(8, *out_avals[i].shape)[c]
             for i, nm in enumerate(out_names)}
            for c in range(8)
        ]

    _CACHED["runner"] = run
    return run


def kernel(x, w_attn, b_attn, w_proj, b_proj):
    x = np.asarray(x, dtype=np.float32)
    w_attn = np.asarray(w_attn, dtype=np.float32)
    b_attn = np.asarray(b_attn, dtype=np.float32)
    w_proj = np.asarray(w_proj, dtype=np.float32)
    b_proj = np.asarray(b_proj, dtype=np.float32)

    in_maps = make_in_maps(x, w_attn, b_attn, w_proj)
    try:
        run = _get_runner()
        results = run(in_maps)
    except Exception:
        # fallback: the stock SPMD runner (slower per call, same result)
        if "nc" not in _CACHED:
            _CACHED["nc"] = build_nc()
        res = run_bass_kernel_spmd(
            _CACHED["nc"], in_maps, core_ids=list(range(8)))
        results = res.results

    # v-bias contribution: probs rows sum to 1, so attn += 1 * b_v^T, and
    # (1 b_v^T) @ w_proj = row vector b_v @ w_proj added to every position.
    extra = b_attn[2 * C:] @ w_proj + b_proj  # [C]
    out = np.empty((B, T, C), dtype=np.float32)
    for b in range(B):
        out[b] = (results[2 * b]["y"].astype(np.float32)
                  + results[2 * b + 1]["y"].astype(np.float32) + extra)
    return out



# revision 36
# speedup vs baseline: 1.2415x; 1.0043x over previous
"""Causal multi-head attention (B=4, T=2048, C=1024, H=16) on 8 Trainium2 cores.

Sharding: core c handles batch b = c//2 and heads h0..h0+7 with h0 = (c%2)*8.
Each core computes QKV projection for its head slice, causal attention for its
8 heads, and a partial output projection. Host sums the two partials per batch
and adds the bias terms.

All matmuls run as float32r (full-speed fp32 on the PE, ~2e-4 relative error).

Attention works in the S^T = K Q^T layout ([k, q], k on partitions) so that
softmax normalization needs no cross-partition reduction: an extra all-ones
column appended to V makes the AV matmul emit the softmax row-sums for free,
and the causal mask is added into PSUM with an identity-matmul of a -1e30
bias tile before the exp.
"""

import os
import sys
import numpy as np

sys.path.insert(0, "/opt/trn_rl_repo")

import concourse.bass as bass  # noqa: E402
import concourse.bacc as bacc  # noqa: E402
import concourse.mybir as mybir  # noqa: E402
from concourse.bass_utils import run_bass_kernel_spmd  # noqa: E402
from concourse.tile import TileContext  # noqa: E402

B, T, C, H = 4, 2048, 1024, 16
HD = C // H          # 64 head dim
HPC = 8              # heads per core
P = 128
NT = T // P          # 16 t-chunks of 128
NS = T // 512        # 4 q-strips of 512
KC = C // P          # 8 contraction chunks for QKV
CL = HPC * HD        # 512 local channels per section
F32 = mybir.dt.float32
F32R = mybir.dt.float32r
BF16 = mybir.dt.bfloat16
F8 = mybir.dt.float8e4
DRM = mybir.MatmulPerfMode.DoubleRow
EXPF = mybir.ActivationFunctionType.Exp
MUL = mybir.AluOpType.mult
WS = 64.0            # host-side w_attn scale for fp8 dynamic range

_CACHED = {}


def build_nc():
    nc = bacc.Bacc("TRN2", target_bir_lowering=False, debug=False)

    xth_d = nc.dram_tensor("xth", [C, T], F8, kind="ExternalInput")
    xtl_d = nc.dram_tensor("xtl", [C, T], F8, kind="ExternalInput")
    wqkh_d = nc.dram_tensor("wqkh", [C, 2 * CL], F8, kind="ExternalInput")
    wqkl_d = nc.dram_tensor("wqkl", [C, 2 * CL], F8, kind="ExternalInput")
    wvh_d = nc.dram_tensor("wvh", [C, CL], F8, kind="ExternalInput")
    wvl_d = nc.dram_tensor("wvl", [C, CL], F8, kind="ExternalInput")
    wp_d = nc.dram_tensor("wp", [CL, C], BF16, kind="ExternalInput")
    ident_d = nc.dram_tensor("ident", [P, P], BF16, kind="ExternalInput")
    bqk_d = nc.dram_tensor("bqk", [P, 8], F32, kind="ExternalInput")
    maskb_d = nc.dram_tensor("maskb", [P, P], BF16, kind="ExternalInput")
    y_d = nc.dram_tensor("y", [T, C], BF16, kind="ExternalOutput")

    xth_r = xth_d.ap().rearrange("(kc p) t -> p kc t", p=P)     # [128, 8, 2048]
    xtl_r = xtl_d.ap().rearrange("(kc p) t -> p kc t", p=P)
    wqkh_r = wqkh_d.ap().rearrange("(kc p) c -> p kc c", p=P)   # [128, 8, 1024]
    wqkl_r = wqkl_d.ap().rearrange("(kc p) c -> p kc c", p=P)
    wvh_r = wvh_d.ap().rearrange("(kc p) c -> p kc c", p=P)     # [128, 8, 512]
    wvl_r = wvl_d.ap().rearrange("(kc p) c -> p kc c", p=P)
    wp_r = wp_d.ap().rearrange("(ct p) c -> p ct c", p=P)       # [128, 4, 1024]
    y_r = y_d.ap().rearrange("(tt p) c -> p tt c", p=P)         # [128, 16, 1024]

    SW = 512            # phase-A t-strip width
    EH = HD + 1         # 65: head slot width in v (value cols + ones col)
    scale = float(HD) ** -0.5 / (WS * WS)

    with TileContext(nc) as tc:
      with tc.tile_pool(name="const", bufs=1) as constp:
        with tc.tile_pool(name="qkv_big", bufs=1) as bigp:
            v_sb = bigp.tile([P, NT, HPC * EH], BF16)
            v_heads = v_sb[:].rearrange("p t (h e) -> p t h e", e=EH)
            qkT = bigp.tile([P, 8, T], BF16)  # c-tiles 0-3 = qT, 4-7 = kT

            # ---------------- Phase A: QKV projections ----------------
            # Single pass per 256-wide t-strip: V matmuls then qT/kT matmuls.
            # All DMAs are chunked per contraction block so the PE can start
            # as soon as the first chunks land; wqk chunks stream during the
            # early v work.
            with (
                tc.tile_pool(name="xts", bufs=3) as xtsp,
                tc.tile_pool(name="wqkv", bufs=1) as wqkvp,
                tc.tile_pool(name="ps_a", bufs=3, space="PSUM") as ps_a,
            ):
                wvh_sb = wqkvp.tile([P, KC, CL], F8)
                wvl_sb = wqkvp.tile([P, KC, CL], F8)
                wqkh_sb = wqkvp.tile([P, KC, 2 * CL], F8)
                wqkl_sb = wqkvp.tile([P, KC, 2 * CL], F8)
                xtsh0 = xtsp.tile([P, KC, SW], F8, tag="xh")
                xtsl0 = xtsp.tile([P, KC, SW], F8, tag="xl")
                # strip-0 feeds in half-chunks (kc 0-3 / 4-7) in the exact
                # order the DoubleRow products consume them
                # single ordered queue, sequenced by first-use time:
                # v-part hi feeds, v-part lo, then wqk hi/lo for the qk part
                for hs in (slice(0, 2), slice(2, 8)):
                    nc.sync.dma_start(xtsh0[:, hs, :], xth_r[:, hs, 0:SW])
                    nc.sync.dma_start(wvh_sb[:, hs, :], wvh_r[:, hs, :])
                nc.sync.dma_start(xtsl0[:], xtl_r[:, :, 0:SW])
                nc.sync.dma_start(wvl_sb[:], wvl_r[:])
                bqk = constp.tile([P, 8], F32)
                nc.sync.dma_start(bqk[:], bqk_d[:])
                for h in range(2):
                    hs = slice(4 * h, 4 * h + 4)
                    nc.sync.dma_start(wqkh_sb[:, hs, :], wqkh_r[:, hs, :])
                for h in range(2):
                    hs = slice(4 * h, 4 * h + 4)
                    nc.sync.dma_start(wqkl_sb[:, hs, :], wqkl_r[:, hs, :])
                ident = constp.tile([P, P], BF16)
                maskb = constp.tile([P, P], BF16)

                NKP = KC // 2
                for ts in range(T // SW):
                    if ts == 0:
                        xtsh, xtsl = xtsh0, xtsl0
                    else:
                        xtsh = xtsp.tile([P, KC, SW], F8, tag="xh")
                        xtsl = xtsp.tile([P, KC, SW], F8, tag="xl")
                        nc.sync.dma_start(
                            xtsh[:], xth_r[:, :, ts * SW:(ts + 1) * SW])
                        nc.sync.dma_start(
                            xtsl[:], xtl_r[:, :, ts * SW:(ts + 1) * SW])
                    # v part: [t, c] orientation; x stationary, wv moving.
                    # 3-product compensated fp8 DoubleRow (hh, lh, hl).
                    for tt in range(SW // P):
                        tch = ts * (SW // P) + tt
                        tsl = slice(tt * P, (tt + 1) * P)
                        psv = ps_a.tile([P, CL], F32, tag="psa")
                        for xt_, wv_, first, last in (
                            (xtsh, wvh_sb, True, False),
                            (xtsl, wvh_sb, False, False),
                            (xtsh, wvl_sb, False, True),
                        ):
                            for kp in range(NKP):
                                s = slice(2 * kp, 2 * kp + 2)
                                nc.tensor.matmul(
                                    psv[:], xt_[:, s, tsl], wv_[:, s, :],
                                    start=(first and kp == 0),
                                    stop=(last and kp == NKP - 1),
                                    perf_mode=DRM)
                        nc.scalar.copy(
                            v_heads[:, tch, :, 0:HD],
                            psv[:].rearrange("p (h d) -> p h d", d=HD),
                        )
                    # qT/kT part: [c, t] orientation; w stationary, x moving
                    for ct in range(8):
                        csl = slice(ct * P, (ct + 1) * P)
                        psq = ps_a.tile([P, SW], F32, tag="psq")
                        for wq_, xt_, first, last in (
                            (wqkh_sb, xtsh, True, False),
                            (wqkl_sb, xtsh, False, False),
                            (wqkh_sb, xtsl, False, True),
                        ):
                            for kp in range(NKP):
                                s = slice(2 * kp, 2 * kp + 2)
                                nc.tensor.matmul(
                                    psq[:], wq_[:, s, csl], xt_[:, s, :],
                                    start=(first and kp == 0),
                                    stop=(last and kp == NKP - 1),
                                    perf_mode=DRM)
                        nc.scalar.activation(
                            qkT[:, ct, ts * SW:(ts + 1) * SW],
                            psq[:],
                            mybir.ActivationFunctionType.Identity,
                            bias=bqk[:, ct:ct + 1], scale=1.0,
                        )
                    if ts == 0:
                        # phase-B constants ride behind the early x strips;
                        # the ones column is a memset, not a DMA scatter
                        nc.sync.dma_start(ident[:], ident_d[:])
                        nc.sync.dma_start(maskb[:], maskb_d[:])
                        nc.vector.memset(v_heads[:, :, :, HD], 1.0)

            # ---------------- Phase B: attention + overlapped projection ----
            # Strip-major over q; heads run in pairs sharing a qkT c-tile
            # (rows 0-63 / 64-127 -> different PE row groups). The AV matmuls
            # lag the score matmuls by LAG k-tiles so the PE never waits on
            # the exp. Each pair is normalized right after its AV finishes
            # (fast-reciprocal + gpsimd row broadcast), and the previous
            # strip's output projection is interleaved into the current
            # strip's attention.
            with (
                tc.tile_pool(name="attnT_p", bufs=1) as attnTp,
                tc.tile_pool(name="proj", bufs=1) as projp,
                tc.tile_pool(name="ystage", bufs=4) as ystagep,
            ):
                attnT = attnTp.tile([P, 4, T], BF16)
                wp_sb = projp.tile([P, 4, C], BF16)

                with (
                    tc.tile_pool(name="u_pool", bufs=7) as up,
                    tc.tile_pool(name="attn_small", bufs=2) as smallp,
                    tc.tile_pool(name="attn_one", bufs=1) as small1p,
                    tc.tile_pool(name="o_un", bufs=5) as ounp,
                    tc.tile_pool(name="ps_s", bufs=2, space="PSUM") as ps_s,
                    tc.tile_pool(name="ps_o", bufs=2, space="PSUM") as ps_o,
                ):
                    LAG = 4

                    def proj_tiles(tt, ps_y):
                        for co in range(2):
                            psy = ps_y.tile([P, 512], F32, tag="psy")
                            for ct in range(4):
                                nc.tensor.matmul(
                                    psy[:],
                                    attnT[:, ct, tt * P:(tt + 1) * P],
                                    wp_sb[:, ct, co * 512:(co + 1) * 512],
                                    start=(ct == 0), stop=(ct == 3),
                                )
                            yt = ystagep.tile([P, 512], BF16, tag="yt")
                            nc.vector.tensor_copy(yt[:], psy[:])
                            nc.sync.dma_start(
                                y_r[:, tt, co * 512:(co + 1) * 512], yt[:])

                    nc.sync.dma_start(wp_sb[:], wp_r)
                    from contextlib import ExitStack
                    _stk = ExitStack()
                    ps_x = _stk.enter_context(
                        tc.tile_pool(name="ps_x", bufs=1, space="PSUM"))
                    ps_y = None
                    _nps = 0
                    STRIP_ORDER = (0, 1, 2, 3)
                    for qi, qj in enumerate(STRIP_ORDER):
                        if qi == 1:
                            _stk.close()  # release first-strip extra psum
                            _stk = ExitStack()
                            ps_x = None
                            ps_y = _stk.enter_context(
                                tc.tile_pool(name="ps_y", bufs=2,
                                             space="PSUM"))
                        nk = 4 * (qj + 1)
                        for pr in range(4):  # head pair (2pr, 2pr+1)
                            qct, kct = pr, 4 + pr
                            psoA = ps_o.tile([EH, 512], F32, tag="ps_o")
                            psoB = ps_o.tile([EH, 512], F32, tag="ps_o")
                            u_ring = {}
                            for step in range(nk + LAG):
                                if step < nk:
                                    kt = step
                                    # columns < q0 of a diagonal tile are
                                    # fully masked: skip them in the scores,
                                    # exp, and AV; only the [128,128] block
                                    # at the diagonal needs the -1e30 mask.
                                    q0 = max(0, kt * P - qj * 512)
                                    diag = kt >= 4 * qj
                                    if ps_x is not None and _nps % 3 == 2:
                                        ps = ps_x.tile([P, 2, 512], F32,
                                                       tag="ps_x")
                                    else:
                                        ps = ps_s.tile([P, 2, 512], F32,
                                                       tag="ps_s")
                                    _nps += 1
                                    u = up.tile([P, 2, 512], BF16, tag="u")
                                    u_ring[kt] = u
                                    for hh in range(2):
                                        hp = hh * HD
                                        nc.tensor.matmul(
                                            ps[:, hh, q0:512],
                                            qkT[hp:hp + HD, kct,
                                                kt * P:(kt + 1) * P],
                                            qkT[hp:hp + HD, qct,
                                                qj * 512 + q0:
                                                (qj + 1) * 512],
                                            start=True, stop=not diag,
                                        )
                                    if diag:
                                        for hh in range(2):
                                            nc.tensor.matmul(
                                                ps[:, hh, q0:q0 + P],
                                                ident[:],
                                                maskb[:],
                                                start=False, stop=True,
                                            )
                                    nc.scalar.activation(
                                        u[:, :, q0:512], ps[:, :, q0:512],
                                        EXPF, scale=scale,
                                    )
                                if step >= LAG:
                                    kt = step - LAG
                                    u = u_ring.pop(kt)
                                    q0 = max(0, kt * P - qj * 512)
                                    last = kt == nk - 1
                                    nc.tensor.matmul(
                                        psoA[0:EH, q0:512],
                                        v_sb[:, kt,
                                             (2 * pr) * EH:(2 * pr + 1) * EH],
                                        u[:, 0, q0:512],
                                        start=(kt == 0), stop=last,
                                    )
                                    nc.tensor.matmul(
                                        psoB[0:EH, q0:512],
                                        v_sb[:, kt,
                                             (2 * pr + 1) * EH:
                                             (2 * pr + 2) * EH],
                                        u[:, 1, q0:512],
                                        start=(kt == 0), stop=last,
                                    )
                            # per-pair normalize: in-place reciprocal on
                            # the rowsum row (partition 64) + direct gpsimd
                            # broadcast from it -- no DMAs except the odd-head
                            # cross-partition store into attnT rows 64-127.
                            qsl = slice(qj * 512, (qj + 1) * 512)
                            # previous strip's projection: issued first so its
                            # psum-freeing copies sit ahead of the normalize
                            # chain in the DVE queue
                            if qi > 0:
                                proj_tiles(4 * STRIP_ORDER[qi - 1] + pr,
                                           ps_y)
                            ounA = ounp.tile([EH, 512], F32, tag="oun")
                            if qi == NS - 1 and pr == 3:
                                # last pair: ACT is drained of exps; split the
                                # two psum copies across engines
                                nc.scalar.copy(ounA[:], psoA[:])
                            else:
                                nc.vector.tensor_copy(ounA[:], psoA[:])
                            ounB = ounp.tile([EH, 512], F32, tag="oun")
                            nc.vector.tensor_copy(ounB[:], psoB[:])
                            # cross-partition reciprocal p64 -> p0 (DVE allows
                            # partition remaps between aligned bases); the
                            # broadcast source must sit at partition 0 on hw
                            rcA = smallp.tile([1, 512], F32, tag="rc")
                            nc.vector.reciprocal(rcA[:], ounA[HD:EH, :])
                            rcB = smallp.tile([1, 512], F32, tag="rc")
                            nc.vector.reciprocal(rcB[:], ounB[HD:EH, :])
                            bcA = smallp.tile([HD, 512], F32, tag="bc")
                            nc.gpsimd.partition_broadcast(bcA[:], rcA[:])
                            bcB = smallp.tile([HD, 512], F32, tag="bc")
                            nc.gpsimd.partition_broadcast(bcB[:], rcB[:])
                            nc.vector.tensor_tensor(
                                attnT[0:HD, pr, qsl],
                                ounA[0:HD, :], bcA[:], MUL,
                            )
                            # odd head: DVE cross-partition write (0-63 ->
                            # 64-127), aligned partition bases
                            nc.vector.tensor_tensor(
                                attnT[HD:P, pr, qsl],
                                ounB[0:HD, :], bcB[:], MUL,
                            )
                    _stk.close()
                # last strip's projection with a wide psum pool (the
                # attention pools are closed by now) so its first three
                # accumulation steps overlap the final normalize chain
                with tc.tile_pool(name="ps_f", bufs=4, space="PSUM") as ps_f:
                    chains = []
                    for tt in range(4 * STRIP_ORDER[-1],
                                    4 * STRIP_ORDER[-1] + 4):
                        chains.append(
                            (tt, ps_f.tile([P, 1024], F32, tag="psf",
                                           name=f"psf_{tt}")))
                    # stage-major so every chain's first three accumulation
                    # steps can run while the last head pair normalizes; the
                    # last stage goes chain-major so each chain's copy + DMA
                    # launches as soon as its accumulation closes
                    for ct in range(3):
                        for tt, psy in chains:
                            for co in range(2):
                                nc.tensor.matmul(
                                    psy[:, co * 512:(co + 1) * 512],
                                    attnT[:, ct, tt * P:(tt + 1) * P],
                                    wp_sb[:, ct, co * 512:(co + 1) * 512],
                                    start=(ct == 0), stop=False,
                                )
                    for i, (tt, psy) in enumerate(chains):
                        for co in range(2):
                            nc.tensor.matmul(
                                psy[:, co * 512:(co + 1) * 512],
                                attnT[:, 3, tt * P:(tt + 1) * P],
                                wp_sb[:, 3, co * 512:(co + 1) * 512],
                                start=False, stop=True,
                            )
                        yt = ystagep.tile([P, 1024], BF16, tag="ytf")
                        if i % 2 == 0:
                            nc.scalar.copy(yt[:], psy[:])
                        else:
                            nc.vector.tensor_copy(yt[:], psy[:])
                        nc.sync.dma_start(y_r[:, tt, :], yt[:])
    nc.compile()
    return nc


def _host_consts():
    import ml_dtypes
    i_idx = np.arange(P, dtype=np.float32)[:, None]
    j_idx = np.arange(P, dtype=np.float32)[None, :]
    maskb = np.where(j_idx - i_idx >= 0, 0.0, -1e30).astype(ml_dtypes.bfloat16)
    ident = np.eye(P, dtype=ml_dtypes.bfloat16)
    return ident, maskb


def make_in_maps(x, w_attn, b_attn, w_proj):
    import ml_dtypes
    bf16 = ml_dtypes.bfloat16
    f8 = ml_dtypes.float8_e4m3
    ident, maskb = _host_consts()
    in_maps = []
    for c in range(8):
        b = c // 2
        h0 = (c % 2) * HPC
        qcols = slice(h0 * HD, h0 * HD + CL)
        kcols = slice(C + h0 * HD, C + h0 * HD + CL)
        vcols = slice(2 * C + h0 * HD, 2 * C + h0 * HD + CL)
        wqk = np.concatenate([w_attn[:, qcols], w_attn[:, kcols]],
                             axis=1) * np.float32(WS)
        wv = w_attn[:, vcols] * np.float32(WS)
        bqk = (np.concatenate([b_attn[qcols], b_attn[kcols]])
               * np.float32(WS)).reshape(8, P).T
        xt = np.ascontiguousarray(x[b].T)
        xth = xt.astype(f8)
        xtl = (xt - xth.astype(np.float32)).astype(f8)
        wqkh = wqk.astype(f8)
        wqkl = (wqk - wqkh.astype(np.float32)).astype(f8)
        wvh = wv.astype(f8)
        wvl = (wv - wvh.astype(np.float32)).astype(f8)
        in_maps.append({
            "xth": xth, "xtl": np.ascontiguousarray(xtl),
            "wqkh": np.ascontiguousarray(wqkh),
            "wqkl": np.ascontiguousarray(wqkl),
            "wvh": np.ascontiguousarray(wvh),
            "wvl": np.ascontiguousarray(wvl),
            "wp": np.ascontiguousarray(
                (w_proj[h0 * HD:h0 * HD + CL, :]
                 / np.float32(WS)).astype(bf16)),
            "bqk": np.ascontiguousarray(bqk),
            "ident": ident,
            "maskb": maskb,
        })
    return in_maps


def _get_runner():
    """Build the SPMD executor once: a cached jax.jit over 8 cores.

    Mirrors bass2jax.run_bass_via_pjrt but hoists the jit so repeated
    kernel() calls reuse the compiled executable.
    """
    if "runner" in _CACHED:
        return _CACHED["runner"]
    import jax
    import jax.numpy as jnp
    from jax.sharding import Mesh, PartitionSpec
    from jax.experimental.shard_map import shard_map
    from concourse import bass2jax
    import concourse.mybir as mybir_

    nc = _CACHED.get("nc")
    if nc is None:
        nc = _CACHED["nc"] = build_nc()
    bass2jax.install_neuronx_cc_hook()

    partition_name = (nc.partition_id_tensor.name
                      if nc.partition_id_tensor else None)
    in_names, out_names, out_avals, zero_shapes = [], [], [], []
    for alloc in nc.m.functions[0].allocations:
        if not isinstance(alloc, mybir_.MemoryLocationSet):
            continue
        name = alloc.memorylocations[0].name
        if alloc.kind == "ExternalInput":
            if name != partition_name:
                in_names.append(name)
        elif alloc.kind == "ExternalOutput":
            shape = tuple(alloc.tensor_shape)
            dtype = mybir_.dt.np(alloc.dtype)
            out_names.append(name)
            out_avals.append(jax.core.ShapedArray(shape, dtype))
            zero_shapes.append((shape, dtype))
    n_params = len(in_names)
    n_outs = len(out_names)
    all_names = in_names + out_names
    if partition_name is not None:
        all_names = all_names + [partition_name]

    def _body(*args):
        operands = list(args)
        if partition_name is not None:
            operands.append(bass2jax.partition_id_tensor())
        outs = bass2jax._bass_exec_p.bind(
            *operands,
            out_avals=tuple(out_avals),
            in_names=tuple(all_names),
            out_names=tuple(out_names),
            lowering_input_output_aliases=(),
            sim_require_finite=True,
            sim_require_nnan=True,
            nc=nc,
        )
        return tuple(outs)

    devices = jax.devices()[:8]
    mesh = Mesh(np.asarray(devices), ("core",))
    in_specs = (PartitionSpec("core"),) * (n_params + n_outs)
    out_specs = (PartitionSpec("core"),) * n_outs
    donate = tuple(range(n_params, n_params + n_outs))
    sharded = jax.jit(
        shard_map(_body, mesh=mesh, in_specs=in_specs, out_specs=out_specs,
                  check_rep=False),
        donate_argnums=donate, keep_unused=True,
    )

    def run(in_maps):
        concat_in = [
            np.concatenate([np.asarray(in_maps[c][nm]) for c in range(8)],
                           axis=0)
            for nm in in_names
        ]
        concat_zeros = [
            np.zeros((8 * s[0], *s[1:]), dt) for (s, dt) in zero_shapes
        ]
        out_arrs = sharded(*concat_in, *concat_zeros)
        return [
            {nm: np.asarray(out_arrs[i]).reshape(8, *out_avals[i].shape)[c]
             for i, nm in enumerate(out_names)}
            for c in range(8)
        ]

    _CACHED["runner"] = run
    return run


def kernel(x, w_attn, b_attn, w_proj, b_proj):
    x = np.asarray(x, dtype=np.float32)
    w_attn = np.asarray(w_attn, dtype=np.float32)
    b_attn = np.asarray(b_attn, dtype=np.float32)
    w_proj = np.asarray(w_proj, dtype=np.float32)
    b_proj = np.asarray(b_proj, dtype=np.float32)

    in_maps = make_in_maps(x, w_attn, b_attn, w_proj)
    try:
        run = _get_runner()
        results = run(in_maps)
    except Exception:
        # fallback: the stock SPMD runner (slower per call, same result)
        if "nc" not in _CACHED:
            _CACHED["nc"] = build_nc()
        res = run_bass_kernel_spmd(
            _CACHED["nc"], in_maps, core_ids=list(range(8)))
        results = res.results

    # v-bias contribution: probs rows sum to 1, so attn += 1 * b_v^T, and
    # (1 b_v^T) @ w_proj = row vector b_v @ w_proj added to every position.
    extra = b_attn[2 * C:] @ w_proj + b_proj  # [C]
    out = np.empty((B, T, C), dtype=np.float32)
    for b in range(B):
        out[b] = (results[2 * b]["y"].astype(np.float32)
                  + results[2 * b + 1]["y"].astype(np.float32) + extra)
    return out



# revision 39
# speedup vs baseline: 1.2435x; 1.0016x over previous
"""Causal multi-head attention (B=4, T=2048, C=1024, H=16) on 8 Trainium2 cores.

Sharding: core c handles batch b = c//2 and heads h0..h0+7 with h0 = (c%2)*8.
Each core computes QKV projection for its head slice, causal attention for its
8 heads, and a partial output projection. Host sums the two partials per batch
and adds the bias terms.

All matmuls run as float32r (full-speed fp32 on the PE, ~2e-4 relative error).

Attention works in the S^T = K Q^T layout ([k, q], k on partitions) so that
softmax normalization needs no cross-partition reduction: an extra all-ones
column appended to V makes the AV matmul emit the softmax row-sums for free,
and the causal mask is added into PSUM with an identity-matmul of a -1e30
bias tile before the exp.
"""

import os
import sys
import numpy as np

sys.path.insert(0, "/opt/trn_rl_repo")

import concourse.bass as bass  # noqa: E402
import concourse.bacc as bacc  # noqa: E402
import concourse.mybir as mybir  # noqa: E402
from concourse.bass_utils import run_bass_kernel_spmd  # noqa: E402
from concourse.tile import TileContext  # noqa: E402

B, T, C, H = 4, 2048, 1024, 16
HD = C // H          # 64 head dim
HPC = 8              # heads per core
P = 128
NT = T // P          # 16 t-chunks of 128
NS = T // 512        # 4 q-strips of 512
KC = C // P          # 8 contraction chunks for QKV
CL = HPC * HD        # 512 local channels per section
F32 = mybir.dt.float32
F32R = mybir.dt.float32r
BF16 = mybir.dt.bfloat16
F8 = mybir.dt.float8e4
DRM = mybir.MatmulPerfMode.DoubleRow
EXPF = mybir.ActivationFunctionType.Exp
MUL = mybir.AluOpType.mult
WS = 64.0            # host-side w_attn scale for fp8 dynamic range

_CACHED = {}


def build_nc():
    nc = bacc.Bacc("TRN2", target_bir_lowering=False, debug=False)

    xth_d = nc.dram_tensor("xth", [C, T], F8, kind="ExternalInput")
    xtl_d = nc.dram_tensor("xtl", [C, T], F8, kind="ExternalInput")
    wqkh_d = nc.dram_tensor("wqkh", [C, 2 * CL], F8, kind="ExternalInput")
    wqkl_d = nc.dram_tensor("wqkl", [C, 2 * CL], F8, kind="ExternalInput")
    wvh_d = nc.dram_tensor("wvh", [C, CL], F8, kind="ExternalInput")
    wvl_d = nc.dram_tensor("wvl", [C, CL], F8, kind="ExternalInput")
    wp_d = nc.dram_tensor("wp", [CL, C], BF16, kind="ExternalInput")
    ident_d = nc.dram_tensor("ident", [P, P], BF16, kind="ExternalInput")
    bqk_d = nc.dram_tensor("bqk", [P, 8], F32, kind="ExternalInput")
    maskb_d = nc.dram_tensor("maskb", [P, P], BF16, kind="ExternalInput")
    y_d = nc.dram_tensor("y", [T, C], BF16, kind="ExternalOutput")

    xth_r = xth_d.ap().rearrange("(kc p) t -> p kc t", p=P)     # [128, 8, 2048]
    xtl_r = xtl_d.ap().rearrange("(kc p) t -> p kc t", p=P)
    wqkh_r = wqkh_d.ap().rearrange("(kc p) c -> p kc c", p=P)   # [128, 8, 1024]
    wqkl_r = wqkl_d.ap().rearrange("(kc p) c -> p kc c", p=P)
    wvh_r = wvh_d.ap().rearrange("(kc p) c -> p kc c", p=P)     # [128, 8, 512]
    wvl_r = wvl_d.ap().rearrange("(kc p) c -> p kc c", p=P)
    wp_r = wp_d.ap().rearrange("(ct p) c -> p ct c", p=P)       # [128, 4, 1024]
    y_r = y_d.ap().rearrange("(tt p) c -> p tt c", p=P)         # [128, 16, 1024]

    SW = 512            # phase-A t-strip width
    EH = HD + 1         # 65: head slot width in v (value cols + ones col)
    scale = float(HD) ** -0.5 / (WS * WS)

    with TileContext(nc) as tc:
      with tc.tile_pool(name="const", bufs=1) as constp:
        with tc.tile_pool(name="qkv_big", bufs=1) as bigp:
            v_sb = bigp.tile([P, NT, HPC * EH], BF16)
            v_heads = v_sb[:].rearrange("p t (h e) -> p t h e", e=EH)
            qkT = bigp.tile([P, 8, T], BF16)  # c-tiles 0-3 = qT, 4-7 = kT

            # ---------------- Phase A: QKV projections ----------------
            # Single pass per 256-wide t-strip: V matmuls then qT/kT matmuls.
            # All DMAs are chunked per contraction block so the PE can start
            # as soon as the first chunks land; wqk chunks stream during the
            # early v work.
            with (
                tc.tile_pool(name="xts", bufs=3) as xtsp,
                tc.tile_pool(name="wqkv", bufs=1) as wqkvp,
                tc.tile_pool(name="ps_a", bufs=3, space="PSUM") as ps_a,
            ):
                wvh_sb = wqkvp.tile([P, KC, CL], F8)
                wvl_sb = wqkvp.tile([P, KC, CL], F8)
                wqkh_sb = wqkvp.tile([P, KC, 2 * CL], F8)
                wqkl_sb = wqkvp.tile([P, KC, 2 * CL], F8)
                xtsh0 = xtsp.tile([P, KC, SW], F8, tag="xh")
                xtsl0 = xtsp.tile([P, KC, SW], F8, tag="xl")
                # strip-0 feeds in half-chunks (kc 0-3 / 4-7) in the exact
                # order the DoubleRow products consume them
                # single ordered queue, sequenced by first-use time:
                # v-part hi feeds, v-part lo, then wqk hi/lo for the qk part
                for hs in (slice(0, 2), slice(2, 8)):
                    nc.sync.dma_start(xtsh0[:, hs, :], xth_r[:, hs, 0:SW])
                    nc.sync.dma_start(wvh_sb[:, hs, :], wvh_r[:, hs, :])
                nc.sync.dma_start(xtsl0[:], xtl_r[:, :, 0:SW])
                nc.sync.dma_start(wvl_sb[:], wvl_r[:])
                bqk = constp.tile([P, 8], F32)
                nc.sync.dma_start(bqk[:], bqk_d[:])
                for h in range(2):
                    hs = slice(4 * h, 4 * h + 4)
                    nc.sync.dma_start(wqkh_sb[:, hs, :], wqkh_r[:, hs, :])
                for h in range(2):
                    hs = slice(4 * h, 4 * h + 4)
                    nc.sync.dma_start(wqkl_sb[:, hs, :], wqkl_r[:, hs, :])
                ident = constp.tile([P, P], BF16)
                maskb = constp.tile([P, P], BF16)

                NKP = KC // 2
                for ts in range(T // SW):
                    if ts == 0:
                        xtsh, xtsl = xtsh0, xtsl0
                    else:
                        xtsh = xtsp.tile([P, KC, SW], F8, tag="xh")
                        xtsl = xtsp.tile([P, KC, SW], F8, tag="xl")
                        nc.sync.dma_start(
                            xtsh[:], xth_r[:, :, ts * SW:(ts + 1) * SW])
                        nc.sync.dma_start(
                            xtsl[:], xtl_r[:, :, ts * SW:(ts + 1) * SW])
                    # v part: [t, c] orientation; x stationary, wv moving.
                    # 3-product compensated fp8 DoubleRow (hh, lh, hl).
                    for tt in range(SW // P):
                        tch = ts * (SW // P) + tt
                        tsl = slice(tt * P, (tt + 1) * P)
                        psv = ps_a.tile([P, CL], F32, tag="psa")
                        for xt_, wv_, first, last in (
                            (xtsh, wvh_sb, True, False),
                            (xtsl, wvh_sb, False, False),
                            (xtsh, wvl_sb, False, True),
                        ):
                            for kp in range(NKP):
                                s = slice(2 * kp, 2 * kp + 2)
                                nc.tensor.matmul(
                                    psv[:], xt_[:, s, tsl], wv_[:, s, :],
                                    start=(first and kp == 0),
                                    stop=(last and kp == NKP - 1),
                                    perf_mode=DRM)
                        nc.scalar.copy(
                            v_heads[:, tch, :, 0:HD],
                            psv[:].rearrange("p (h d) -> p h d", d=HD),
                        )
                    # qT/kT part: [c, t] orientation; w stationary, x moving
                    for ct in range(8):
                        csl = slice(ct * P, (ct + 1) * P)
                        psq = ps_a.tile([P, SW], F32, tag="psq")
                        for wq_, xt_, first, last in (
                            (wqkh_sb, xtsh, True, False),
                            (wqkl_sb, xtsh, False, False),
                            (wqkh_sb, xtsl, False, True),
                        ):
                            for kp in range(NKP):
                                s = slice(2 * kp, 2 * kp + 2)
                                nc.tensor.matmul(
                                    psq[:], wq_[:, s, csl], xt_[:, s, :],
                                    start=(first and kp == 0),
                                    stop=(last and kp == NKP - 1),
                                    perf_mode=DRM)
                        if ts == T // SW - 1 and ct % 2 == 1:
                            # split the final strip's psum drains across
                            # engines so phase B's psum isn't gated on ACT
                            nc.vector.tensor_scalar_add(
                                qkT[:, ct, ts * SW:(ts + 1) * SW],
                                psq[:],
                                bqk[:, ct:ct + 1],
                            )
                        else:
                            nc.scalar.activation(
                                qkT[:, ct, ts * SW:(ts + 1) * SW],
                                psq[:],
                                mybir.ActivationFunctionType.Identity,
                                bias=bqk[:, ct:ct + 1], scale=1.0,
                            )
                    if ts == 0:
                        # phase-B constants ride behind the early x strips;
                        # the ones column is a memset, not a DMA scatter
                        nc.sync.dma_start(ident[:], ident_d[:])
                        nc.sync.dma_start(maskb[:], maskb_d[:])
                        nc.vector.memset(v_heads[:, :, :, HD], 1.0)

            # ---------------- Phase B: attention + overlapped projection ----
            # Strip-major over q; heads run in pairs sharing a qkT c-tile
            # (rows 0-63 / 64-127 -> different PE row groups). The AV matmuls
            # lag the score matmuls by LAG k-tiles so the PE never waits on
            # the exp. Each pair is normalized right after its AV finishes
            # (fast-reciprocal + gpsimd row broadcast), and the previous
            # strip's output projection is interleaved into the current
            # strip's attention.
            with (
                tc.tile_pool(name="attnT_p", bufs=1) as attnTp,
                tc.tile_pool(name="proj", bufs=1) as projp,
                tc.tile_pool(name="ystage", bufs=6) as ystagep,
            ):
                attnT = attnTp.tile([P, 4, T], BF16)
                wp_sb = projp.tile([P, 4, C], BF16)

                with (
                    tc.tile_pool(name="u_pool", bufs=7) as up,
                    tc.tile_pool(name="attn_small", bufs=3) as smallp,
                    tc.tile_pool(name="attn_one", bufs=1) as small1p,
                    tc.tile_pool(name="o_un", bufs=6) as ounp,
                    tc.tile_pool(name="ps_s", bufs=2, space="PSUM") as ps_s,
                    tc.tile_pool(name="ps_o", bufs=2, space="PSUM") as ps_o,
                ):
                    LAG = 4

                    def proj_chain(tt, co, ps_y):
                        psy = ps_y.tile([P, 512], F32, tag="psy")
                        for ct in range(4):
                            nc.tensor.matmul(
                                psy[:],
                                attnT[:, ct, tt * P:(tt + 1) * P],
                                wp_sb[:, ct, co * 512:(co + 1) * 512],
                                start=(ct == 0), stop=(ct == 3),
                            )
                        yt = ystagep.tile([P, 512], BF16, tag="yt")
                        nc.vector.tensor_copy(yt[:], psy[:])
                        nc.sync.dma_start(
                            y_r[:, tt, co * 512:(co + 1) * 512], yt[:])

                    nc.sync.dma_start(wp_sb[:], wp_r)
                    from contextlib import ExitStack
                    _stk = ExitStack()
                    ps_x = _stk.enter_context(
                        tc.tile_pool(name="ps_x", bufs=1, space="PSUM"))
                    ps_y = None
                    _nps = 0
                    # projection chains for a completed strip, consumed at a
                    # rate matched to each strip's exp-vs-PE deficit (the
                    # later strips are activation-bound and need more PE
                    # filler per pair)
                    proj_fifo = []
                    PROJ_PER_PAIR = {0: [0, 0, 0, 0], 1: [1, 1, 1, 1],
                                     2: [2, 2, 2, 2], 3: [2, 3, 3, 4]}
                    STRIP_ORDER = (0, 1, 2, 3)
                    for qi, qj in enumerate(STRIP_ORDER):
                        if qi == 1:
                            _stk.close()  # release first-strip extra psum
                            _stk = ExitStack()
                            ps_x = None
                            ps_y = _stk.enter_context(
                                tc.tile_pool(name="ps_y", bufs=2,
                                             space="PSUM"))
                        nk = 4 * (qj + 1)
                        for pr in range(4):  # head pair (2pr, 2pr+1)
                            qct, kct = pr, 4 + pr
                            psoA = ps_o.tile([EH, 512], F32, tag="ps_o")
                            psoB = ps_o.tile([EH, 512], F32, tag="ps_o")
                            u_ring = {}
                            for step in range(nk + LAG):
                                if step < nk:
                                    kt = step
                                    # columns < q0 of a diagonal tile are
                                    # fully masked: skip them in the scores,
                                    # exp, and AV; only the [128,128] block
                                    # at the diagonal needs the -1e30 mask.
                                    q0 = max(0, kt * P - qj * 512)
                                    diag = kt >= 4 * qj
                                    if ps_x is not None and _nps % 3 == 2:
                                        ps = ps_x.tile([P, 2, 512], F32,
                                                       tag="ps_x")
                                    else:
                                        ps = ps_s.tile([P, 2, 512], F32,
                                                       tag="ps_s")
                                    _nps += 1
                                    u = up.tile([P, 2, 512], BF16, tag="u")
                                    u_ring[kt] = u
                                    for hh in range(2):
                                        hp = hh * HD
                                        nc.tensor.matmul(
                                            ps[:, hh, q0:512],
                                            qkT[hp:hp + HD, kct,
                                                kt * P:(kt + 1) * P],
                                            qkT[hp:hp + HD, qct,
                                                qj * 512 + q0:
                                                (qj + 1) * 512],
                                            start=True, stop=not diag,
                                        )
                                    if diag:
                                        for hh in range(2):
                                            nc.tensor.matmul(
                                                ps[:, hh, q0:q0 + P],
                                                ident[:],
                                                maskb[:],
                                                start=False, stop=True,
                                            )
                                    nc.scalar.activation(
                                        u[:, :, q0:512], ps[:, :, q0:512],
                                        EXPF, scale=scale,
                                    )
                                if step >= LAG:
                                    kt = step - LAG
                                    u = u_ring.pop(kt)
                                    q0 = max(0, kt * P - qj * 512)
                                    last = kt == nk - 1
                                    nc.tensor.matmul(
                                        psoA[0:EH, q0:512],
                                        v_sb[:, kt,
                                             (2 * pr) * EH:(2 * pr + 1) * EH],
                                        u[:, 0, q0:512],
                                        start=(kt == 0), stop=last,
                                    )
                                    nc.tensor.matmul(
                                        psoB[0:EH, q0:512],
                                        v_sb[:, kt,
                                             (2 * pr + 1) * EH:
                                             (2 * pr + 2) * EH],
                                        u[:, 1, q0:512],
                                        start=(kt == 0), stop=last,
                                    )
                            # per-pair normalize: in-place reciprocal on
                            # the rowsum row (partition 64) + direct gpsimd
                            # broadcast from it -- no DMAs except the odd-head
                            # cross-partition store into attnT rows 64-127.
                            qsl = slice(qj * 512, (qj + 1) * 512)
                            # previous strip's projection: issued first so its
                            # psum-freeing copies sit ahead of the normalize
                            # chain in the DVE queue
                            for _ in range(PROJ_PER_PAIR[qi][pr]):
                                if proj_fifo:
                                    t_, c_ = proj_fifo.pop(0)
                                    proj_chain(t_, c_, ps_y)
                            ounA = ounp.tile([EH, 512], F32, tag="oun")
                            if qi == NS - 1 and pr == 3:
                                # last pair: ACT is drained of exps; split the
                                # two psum copies across engines
                                nc.scalar.copy(ounA[:], psoA[:])
                            else:
                                nc.vector.tensor_copy(ounA[:], psoA[:])
                            ounB = ounp.tile([EH, 512], F32, tag="oun")
                            nc.vector.tensor_copy(ounB[:], psoB[:])
                            # cross-partition reciprocal p64 -> p0 (DVE allows
                            # partition remaps between aligned bases); the
                            # broadcast source must sit at partition 0 on hw
                            rcA = smallp.tile([1, 512], F32, tag="rc")
                            nc.vector.reciprocal(rcA[:], ounA[HD:EH, :])
                            rcB = smallp.tile([1, 512], F32, tag="rc")
                            nc.vector.reciprocal(rcB[:], ounB[HD:EH, :])
                            bcA = smallp.tile([HD, 512], F32, tag="bc")
                            nc.gpsimd.partition_broadcast(bcA[:], rcA[:])
                            bcB = smallp.tile([HD, 512], F32, tag="bc")
                            nc.gpsimd.partition_broadcast(bcB[:], rcB[:])
                            nc.vector.tensor_tensor(
                                attnT[0:HD, pr, qsl],
                                ounA[0:HD, :], bcA[:], MUL,
                            )
                            # odd head: DVE cross-partition write (0-63 ->
                            # 64-127), aligned partition bases
                            nc.vector.tensor_tensor(
                                attnT[HD:P, pr, qsl],
                                ounB[0:HD, :], bcB[:], MUL,
                            )
                        for t_ in range(4 * qj, 4 * qj + 4):
                            for c_ in range(2):
                                proj_fifo.append((t_, c_))
                    _stk.close()
                # last strip's projection with a wide psum pool (the
                # attention pools are closed by now) so its first three
                # accumulation steps overlap the final normalize chain
                with tc.tile_pool(name="ps_f", bufs=4, space="PSUM") as ps_f:
                    chains = []
                    for tt in range(4 * STRIP_ORDER[-1],
                                    4 * STRIP_ORDER[-1] + 4):
                        chains.append(
                            (tt, ps_f.tile([P, 1024], F32, tag="psf",
                                           name=f"psf_{tt}")))
                    # stage-major so every chain's first three accumulation
                    # steps can run while the last head pair normalizes; the
                    # last stage goes chain-major so each chain's copy + DMA
                    # launches as soon as its accumulation closes
                    for ct in range(3):
                        for tt, psy in chains:
                            for co in range(2):
                                nc.tensor.matmul(
                                    psy[:, co * 512:(co + 1) * 512],
                                    attnT[:, ct, tt * P:(tt + 1) * P],
                                    wp_sb[:, ct, co * 512:(co + 1) * 512],
                                    start=(ct == 0), stop=False,
                                )
                    for i, (tt, psy) in enumerate(chains):
                        for co in range(2):
                            nc.tensor.matmul(
                                psy[:, co * 512:(co + 1) * 512],
                                attnT[:, 3, tt * P:(tt + 1) * P],
                                wp_sb[:, 3, co * 512:(co + 1) * 512],
                                start=False, stop=True,
                            )
                        yt = ystagep.tile([P, 1024], BF16, tag="ytf")
                        if i % 2 == 0:
                            nc.scalar.copy(yt[:], psy[:])
                        else:
                            nc.vector.tensor_copy(yt[:], psy[:])
                        nc.sync.dma_start(y_r[:, tt, :], yt[:])
    nc.compile()
    return nc


def _host_consts():
    import ml_dtypes
    i_idx = np.arange(P, dtype=np.float32)[:, None]
    j_idx = np.arange(P, dtype=np.float32)[None, :]
    maskb = np.where(j_idx - i_idx >= 0, 0.0, -1e30).astype(ml_dtypes.bfloat16)
    ident = np.eye(P, dtype=ml_dtypes.bfloat16)
    return ident, maskb


def make_in_maps(x, w_attn, b_attn, w_proj):
    import ml_dtypes
    bf16 = ml_dtypes.bfloat16
    f8 = ml_dtypes.float8_e4m3
    ident, maskb = _host_consts()
    in_maps = []
    for c in range(8):
        b = c // 2
        h0 = (c % 2) * HPC
        qcols = slice(h0 * HD, h0 * HD + CL)
        kcols = slice(C + h0 * HD, C + h0 * HD + CL)
        vcols = slice(2 * C + h0 * HD, 2 * C + h0 * HD + CL)
        wqk = np.concatenate([w_attn[:, qcols], w_attn[:, kcols]],
                             axis=1) * np.float32(WS)
        wv = w_attn[:, vcols] * np.float32(WS)
        bqk = (np.concatenate([b_attn[qcols], b_attn[kcols]])
               * np.float32(WS)).reshape(8, P).T
        xt = np.ascontiguousarray(x[b].T)
        xth = xt.astype(f8)
        xtl = (xt - xth.astype(np.float32)).astype(f8)
        wqkh = wqk.astype(f8)
        wqkl = (wqk - wqkh.astype(np.float32)).astype(f8)
        wvh = wv.astype(f8)
        wvl = (wv - wvh.astype(np.float32)).astype(f8)
        in_maps.append({
            "xth": xth, "xtl": np.ascontiguousarray(xtl),
            "wqkh": np.ascontiguousarray(wqkh),
            "wqkl": np.ascontiguousarray(wqkl),
            "wvh": np.ascontiguousarray(wvh),
            "wvl": np.ascontiguousarray(wvl),
            "wp": np.ascontiguousarray(
                (w_proj[h0 * HD:h0 * HD + CL, :]
                 / np.float32(WS)).astype(bf16)),
            "bqk": np.ascontiguousarray(bqk),
            "ident": ident,
            "maskb": maskb,
        })
    return in_maps


def _get_runner():
    """Build the SPMD executor once: a cached jax.jit over 8 cores.

    Mirrors bass2jax.run_bass_via_pjrt but hoists the jit so repeated
    kernel() calls reuse the compiled executable.
    """
    if "runner" in _CACHED:
        return _CACHED["runner"]
    import jax
    import jax.numpy as jnp
    from jax.sharding import Mesh, PartitionSpec
    from jax.experimental.shard_map import shard_map
    from concourse import bass2jax
    import concourse.mybir as mybir_

    nc = _CACHED.get("nc")
    if nc is None:
        nc = _CACHED["nc"] = build_nc()
    bass2jax.install_neuronx_cc_hook()

    partition_name = (nc.partition_id_tensor.name
                      if nc.partition_id_tensor else None)
    in_names, out_names, out_avals, zero_shapes = [], [], [], []
    for alloc in nc.m.functions[0].allocations:
        if not isinstance(alloc, mybir_.MemoryLocationSet):
            continue
        name = alloc.memorylocations[0].name
        if alloc.kind == "ExternalInput":
            if name != partition_name:
                in_names.append(name)
        elif alloc.kind == "ExternalOutput":
            shape = tuple(alloc.tensor_shape)
            dtype = mybir_.dt.np(alloc.dtype)
            out_names.append(name)
            out_avals.append(jax.core.ShapedArray(shape, dtype))
            zero_shapes.append((shape, dtype))
    n_params = len(in_names)
    n_outs = len(out_names)
    all_names = in_names + out_names
    if partition_name is not None:
        all_names = all_names + [partition_name]

    def _body(*args):
        operands = list(args)
        if partition_name is not None:
            operands.append(bass2jax.partition_id_tensor())
        outs = bass2jax._bass_exec_p.bind(
            *operands,
            out_avals=tuple(out_avals),
            in_names=tuple(all_names),
            out_names=tuple(out_names),
            lowering_input_output_aliases=(),
            sim_require_finite=True,
            sim_require_nnan=True,
            nc=nc,
        )
        return tuple(outs)

    devices = jax.devices()[:8]
    mesh = Mesh(np.asarray(devices), ("core",))
    in_specs = (PartitionSpec("core"),) * (n_params + n_outs)
    out_specs = (PartitionSpec("core"),) * n_outs
    donate = tuple(range(n_params, n_params + n_outs))
    sharded = jax.jit(
        shard_map(_body, mesh=mesh, in_specs=in_specs, out_specs=out_specs,
                  check_rep=False),
        donate_argnums=donate, keep_unused=True,
    )

    def run(in_maps):
        concat_in = [
            np.concatenate([np.asarray(in_maps[c][nm]) for c in range(8)],
                           axis=0)
            for nm in in_names
        ]
        concat_zeros = [
            np.zeros((8 * s[0], *s[1:]), dt) for (s, dt) in zero_shapes
        ]
        out_arrs = sharded(*concat_in, *concat_zeros)
        return [
            {nm: np.asarray(out_arrs[i]).reshape(8, *out_avals[i].shape)[c]
             for i, nm in enumerate(out_names)}
            for c in range(8)
        ]

    _CACHED["runner"] = run
    return run


def kernel(x, w_attn, b_attn, w_proj, b_proj):
    x = np.asarray(x, dtype=np.float32)
    w_attn = np.asarray(w_attn, dtype=np.float32)
    b_attn = np.asarray(b_attn, dtype=np.float32)
    w_proj = np.asarray(w_proj, dtype=np.float32)
    b_proj = np.asarray(b_proj, dtype=np.float32)

    in_maps = make_in_maps(x, w_attn, b_attn, w_proj)
    try:
        run = _get_runner()
        results = run(in_maps)
    except Exception:
        # fallback: the stock SPMD runner (slower per call, same result)
        if "nc" not in _CACHED:
            _CACHED["nc"] = build_nc()
        res = run_bass_kernel_spmd(
            _CACHED["nc"], in_maps, core_ids=list(range(8)))
        results = res.results

    # v-bias contribution: probs rows sum to 1, so attn += 1 * b_v^T, and
    # (1 b_v^T) @ w_proj = row vector b_v @ w_proj added to every position.
    extra = b_attn[2 * C:] @ w_proj + b_proj  # [C]
    out = np.empty((B, T, C), dtype=np.float32)
    for b in range(B):
        out[b] = (results[2 * b]["y"].astype(np.float32)
                  + results[2 * b + 1]["y"].astype(np.float32) + extra)
    return out



# revision 41
# speedup vs baseline: 1.2597x; 1.0131x over previous
"""Causal multi-head attention (B=4, T=2048, C=1024, H=16) on 8 Trainium2 cores.

Sharding: core c handles batch b = c//2 and heads h0..h0+7 with h0 = (c%2)*8.
Each core computes QKV projection for its head slice, causal attention for its
8 heads, and a partial output projection. Host sums the two partials per batch
and adds the bias terms.

All matmuls run as float32r (full-speed fp32 on the PE, ~2e-4 relative error).

Attention works in the S^T = K Q^T layout ([k, q], k on partitions) so that
softmax normalization needs no cross-partition reduction: an extra all-ones
column appended to V makes the AV matmul emit the softmax row-sums for free,
and the causal mask is added into PSUM with an identity-matmul of a -1e30
bias tile before the exp.
"""

import os
import sys
import numpy as np

sys.path.insert(0, "/opt/trn_rl_repo")

import concourse.bass as bass  # noqa: E402
import concourse.bacc as bacc  # noqa: E402
import concourse.mybir as mybir  # noqa: E402
from concourse.bass_utils import run_bass_kernel_spmd  # noqa: E402
from concourse.tile import TileContext  # noqa: E402

B, T, C, H = 4, 2048, 1024, 16
HD = C // H          # 64 head dim
HPC = 8              # heads per core
P = 128
NT = T // P          # 16 t-chunks of 128
NS = T // 512        # 4 q-strips of 512
KC = C // P          # 8 contraction chunks for QKV
CL = HPC * HD        # 512 local channels per section
F32 = mybir.dt.float32
F32R = mybir.dt.float32r
BF16 = mybir.dt.bfloat16
F8 = mybir.dt.float8e4
DRM = mybir.MatmulPerfMode.DoubleRow
EXPF = mybir.ActivationFunctionType.Exp
MUL = mybir.AluOpType.mult
WS = 64.0            # host-side w_attn scale for fp8 dynamic range

_CACHED = {}


def build_nc():
    nc = bacc.Bacc("TRN2", target_bir_lowering=False, debug=False)

    xth_d = nc.dram_tensor("xth", [C, T], F8, kind="ExternalInput")
    xtl_d = nc.dram_tensor("xtl", [C, T], F8, kind="ExternalInput")
    wqkh_d = nc.dram_tensor("wqkh", [C, 2 * CL], F8, kind="ExternalInput")
    wqkl_d = nc.dram_tensor("wqkl", [C, 2 * CL], F8, kind="ExternalInput")
    wvh_d = nc.dram_tensor("wvh", [C, CL], F8, kind="ExternalInput")
    wvl_d = nc.dram_tensor("wvl", [C, CL], F8, kind="ExternalInput")
    wp_d = nc.dram_tensor("wp", [CL, C], BF16, kind="ExternalInput")
    ident_d = nc.dram_tensor("ident", [P, P], BF16, kind="ExternalInput")
    bqk_d = nc.dram_tensor("bqk", [P, 8], F32, kind="ExternalInput")
    maskb_d = nc.dram_tensor("maskb", [P, P], BF16, kind="ExternalInput")
    y_d = nc.dram_tensor("y", [T, C], BF16, kind="ExternalOutput")

    xth_r = xth_d.ap().rearrange("(kc p) t -> p kc t", p=P)     # [128, 8, 2048]
    xtl_r = xtl_d.ap().rearrange("(kc p) t -> p kc t", p=P)
    wqkh_r = wqkh_d.ap().rearrange("(kc p) c -> p kc c", p=P)   # [128, 8, 1024]
    wqkl_r = wqkl_d.ap().rearrange("(kc p) c -> p kc c", p=P)
    wvh_r = wvh_d.ap().rearrange("(kc p) c -> p kc c", p=P)     # [128, 8, 512]
    wvl_r = wvl_d.ap().rearrange("(kc p) c -> p kc c", p=P)
    wp_r = wp_d.ap().rearrange("(ct p) c -> p ct c", p=P)       # [128, 4, 1024]
    y_r = y_d.ap().rearrange("(tt p) c -> p tt c", p=P)         # [128, 16, 1024]

    SW = 512            # phase-A t-strip width
    EH = HD + 1         # 65: head slot width in v (value cols + ones col)
    scale = float(HD) ** -0.5 / (WS * WS)

    with TileContext(nc) as tc:
      with tc.tile_pool(name="const", bufs=1) as constp:
        with (
            tc.tile_pool(name="qkv_big", bufs=1) as bigp,
            tc.tile_pool(name="xts", bufs=3) as xtsp,
            tc.tile_pool(name="wqkv", bufs=1) as wqkvp,
        ):
            v_sb = bigp.tile([P, NT, HPC * EH], BF16)
            v_heads = v_sb[:].rearrange("p t (h e) -> p t h e", e=EH)
            qkT = bigp.tile([P, 8, T], BF16)  # c-tiles 0-3 = qT, 4-7 = kT

            # ---------------- Phase A: QKV projections ----------------
            # Single pass per 256-wide t-strip: V matmuls then qT/kT matmuls.
            # All DMAs are chunked per contraction block so the PE can start
            # as soon as the first chunks land; wqk chunks stream during the
            # early v work.
            with (
                tc.tile_pool(name="ps_a", bufs=3, space="PSUM") as ps_a,
            ):
                wvh_sb = wqkvp.tile([P, KC, CL], F8)
                wvl_sb = wqkvp.tile([P, KC, CL], F8)
                wqkh_sb = wqkvp.tile([P, KC, 2 * CL], F8)
                wqkl_sb = wqkvp.tile([P, KC, 2 * CL], F8)
                xtsh0 = xtsp.tile([P, KC, SW], F8, tag="xh")
                xtsl0 = xtsp.tile([P, KC, SW], F8, tag="xl")
                # strip-0 feeds in half-chunks (kc 0-3 / 4-7) in the exact
                # order the DoubleRow products consume them
                # single ordered queue, sequenced by first-use time:
                # v-part hi feeds, v-part lo, then wqk hi/lo for the qk part
                for hs in (slice(0, 2), slice(2, 8)):
                    nc.sync.dma_start(xtsh0[:, hs, :], xth_r[:, hs, 0:SW])
                    nc.sync.dma_start(wvh_sb[:, hs, :], wvh_r[:, hs, :])
                nc.sync.dma_start(xtsl0[:], xtl_r[:, :, 0:SW])
                nc.sync.dma_start(wvl_sb[:], wvl_r[:])
                bqk = constp.tile([P, 8], F32)
                nc.sync.dma_start(bqk[:], bqk_d[:])
                for h in range(2):
                    hs = slice(4 * h, 4 * h + 4)
                    nc.sync.dma_start(wqkh_sb[:, hs, :], wqkh_r[:, hs, :])
                for h in range(2):
                    hs = slice(4 * h, 4 * h + 4)
                    nc.sync.dma_start(wqkl_sb[:, hs, :], wqkl_r[:, hs, :])
                ident = constp.tile([P, P], BF16)
                maskb = constp.tile([P, P], BF16)

                NKP = KC // 2
                xts3 = None
                for ts in range(T // SW):
                    if ts == 0:
                        xtsh, xtsl = xtsh0, xtsl0
                    else:
                        xtsh = xtsp.tile([P, KC, SW], F8, tag="xh")
                        xtsl = xtsp.tile([P, KC, SW], F8, tag="xl")
                        nc.sync.dma_start(
                            xtsh[:], xth_r[:, :, ts * SW:(ts + 1) * SW])
                        nc.sync.dma_start(
                            xtsl[:], xtl_r[:, :, ts * SW:(ts + 1) * SW])
                    # v part: [t, c] orientation; x stationary, wv moving.
                    # 3-product compensated fp8 DoubleRow (hh, lh, hl).
                    for tt in range(SW // P):
                        tch = ts * (SW // P) + tt
                        tsl = slice(tt * P, (tt + 1) * P)
                        psv = ps_a.tile([P, CL], F32, tag="psa")
                        for xt_, wv_, first, last in (
                            (xtsh, wvh_sb, True, False),
                            (xtsl, wvh_sb, False, False),
                            (xtsh, wvl_sb, False, True),
                        ):
                            for kp in range(NKP):
                                s = slice(2 * kp, 2 * kp + 2)
                                nc.tensor.matmul(
                                    psv[:], xt_[:, s, tsl], wv_[:, s, :],
                                    start=(first and kp == 0),
                                    stop=(last and kp == NKP - 1),
                                    perf_mode=DRM)
                        nc.scalar.copy(
                            v_heads[:, tch, :, 0:HD],
                            psv[:].rearrange("p (h d) -> p h d", d=HD),
                        )
                    if ts == T // SW - 1:
                        # last strip's qk is deferred into phase B strip 0
                        # as PE filler (nothing reads that qkT range until
                        # strip 3); only its v part runs here
                        xts3 = (xtsh, xtsl)
                        continue
                    # qT/kT part: [c, t] orientation; w stationary, x moving
                    for ct in range(8):
                        csl = slice(ct * P, (ct + 1) * P)
                        psq = ps_a.tile([P, SW], F32, tag="psq")
                        for wq_, xt_, first, last in (
                            (wqkh_sb, xtsh, True, False),
                            (wqkl_sb, xtsh, False, False),
                            (wqkh_sb, xtsl, False, True),
                        ):
                            for kp in range(NKP):
                                s = slice(2 * kp, 2 * kp + 2)
                                nc.tensor.matmul(
                                    psq[:], wq_[:, s, csl], xt_[:, s, :],
                                    start=(first and kp == 0),
                                    stop=(last and kp == NKP - 1),
                                    perf_mode=DRM)
                        nc.scalar.activation(
                            qkT[:, ct, ts * SW:(ts + 1) * SW],
                            psq[:],
                            mybir.ActivationFunctionType.Identity,
                            bias=bqk[:, ct:ct + 1], scale=1.0,
                        )
                    if ts == 0:
                        # phase-B constants ride behind the early x strips;
                        # the ones column is a memset, not a DMA scatter
                        nc.sync.dma_start(ident[:], ident_d[:])
                        nc.sync.dma_start(maskb[:], maskb_d[:])
                        nc.vector.memset(v_heads[:, :, :, HD], 1.0)

            # ---------------- Phase B: attention + overlapped projection ----
            # Strip-major over q; heads run in pairs sharing a qkT c-tile
            # (rows 0-63 / 64-127 -> different PE row groups). The AV matmuls
            # lag the score matmuls by LAG k-tiles so the PE never waits on
            # the exp. Each pair is normalized right after its AV finishes
            # (fast-reciprocal + gpsimd row broadcast), and the previous
            # strip's output projection is interleaved into the current
            # strip's attention.
            with (
                tc.tile_pool(name="attnT_p", bufs=1) as attnTp,
                tc.tile_pool(name="proj", bufs=1) as projp,
                tc.tile_pool(name="ystage", bufs=6) as ystagep,
            ):
                attnT = attnTp.tile([P, 4, T], BF16)
                wp_sb = projp.tile([P, 4, C], BF16)

                with (
                    tc.tile_pool(name="u_pool", bufs=7) as up,
                    tc.tile_pool(name="attn_small", bufs=3) as smallp,
                    tc.tile_pool(name="attn_one", bufs=1) as small1p,
                    tc.tile_pool(name="o_un", bufs=6) as ounp,
                    tc.tile_pool(name="ps_s", bufs=2, space="PSUM") as ps_s,
                    tc.tile_pool(name="ps_o", bufs=2, space="PSUM") as ps_o,
                ):
                    LAG = 4

                    def proj_chain(tt, co, ps_y):
                        psy = ps_y.tile([P, 512], F32, tag="psy")
                        for ct in range(4):
                            nc.tensor.matmul(
                                psy[:],
                                attnT[:, ct, tt * P:(tt + 1) * P],
                                wp_sb[:, ct, co * 512:(co + 1) * 512],
                                start=(ct == 0), stop=(ct == 3),
                            )
                        yt = ystagep.tile([P, 512], BF16, tag="yt")
                        nc.vector.tensor_copy(yt[:], psy[:])
                        nc.sync.dma_start(
                            y_r[:, tt, co * 512:(co + 1) * 512], yt[:])

                    nc.sync.dma_start(wp_sb[:], wp_r)
                    from contextlib import ExitStack
                    _stk = ExitStack()
                    ps_qd = _stk.enter_context(
                        tc.tile_pool(name="ps_qd", bufs=2, space="PSUM"))
                    ps_y = None
                    _nps = 0
                    # projection chains for a completed strip, consumed at a
                    # rate matched to each strip's exp-vs-PE deficit (the
                    # later strips are activation-bound and need more PE
                    # filler per pair)
                    proj_fifo = []
                    PROJ_PER_PAIR = {0: [0, 0, 0, 0], 1: [1, 1, 1, 1],
                                     2: [2, 2, 2, 2], 3: [2, 3, 3, 4]}
                    STRIP_ORDER = (0, 1, 2, 3)
                    for qi, qj in enumerate(STRIP_ORDER):
                        if qi == 1:
                            _stk.close()  # release the deferred-qk psum
                            _stk = ExitStack()
                            ps_qd = None
                            ps_y = _stk.enter_context(
                                tc.tile_pool(name="ps_y", bufs=2,
                                             space="PSUM"))
                        nk = 4 * (qj + 1)
                        for pr in range(4):  # head pair (2pr, 2pr+1)
                            qct, kct = pr, 4 + pr
                            psoA = ps_o.tile([EH, 512], F32, tag="ps_o")
                            psoB = ps_o.tile([EH, 512], F32, tag="ps_o")
                            u_ring = {}
                            for step in range(nk + LAG):
                                if step < nk:
                                    kt = step
                                    # columns < q0 of a diagonal tile are
                                    # fully masked: skip them in the scores,
                                    # exp, and AV; only the [128,128] block
                                    # at the diagonal needs the -1e30 mask.
                                    q0 = max(0, kt * P - qj * 512)
                                    diag = kt >= 4 * qj
                                    ps = ps_s.tile([P, 2, 512], F32,
                                                   tag="ps_s")
                                    u = up.tile([P, 2, 512], BF16, tag="u")
                                    u_ring[kt] = u
                                    for hh in range(2):
                                        hp = hh * HD
                                        nc.tensor.matmul(
                                            ps[:, hh, q0:512],
                                            qkT[hp:hp + HD, kct,
                                                kt * P:(kt + 1) * P],
                                            qkT[hp:hp + HD, qct,
                                                qj * 512 + q0:
                                                (qj + 1) * 512],
                                            start=True, stop=not diag,
                                        )
                                    if diag:
                                        for hh in range(2):
                                            nc.tensor.matmul(
                                                ps[:, hh, q0:q0 + P],
                                                ident[:],
                                                maskb[:],
                                                start=False, stop=True,
                                            )
                                    nc.scalar.activation(
                                        u[:, :, q0:512], ps[:, :, q0:512],
                                        EXPF, scale=scale,
                                    )
                                if step >= LAG:
                                    kt = step - LAG
                                    u = u_ring.pop(kt)
                                    q0 = max(0, kt * P - qj * 512)
                                    last = kt == nk - 1
                                    nc.tensor.matmul(
                                        psoA[0:EH, q0:512],
                                        v_sb[:, kt,
                                             (2 * pr) * EH:(2 * pr + 1) * EH],
                                        u[:, 0, q0:512],
                                        start=(kt == 0), stop=last,
                                    )
                                    nc.tensor.matmul(
                                        psoB[0:EH, q0:512],
                                        v_sb[:, kt,
                                             (2 * pr + 1) * EH:
                                             (2 * pr + 2) * EH],
                                        u[:, 1, q0:512],
                                        start=(kt == 0), stop=last,
                                    )
                            # per-pair normalize: in-place reciprocal on
                            # the rowsum row (partition 64) + direct gpsimd
                            # broadcast from it -- no DMAs except the odd-head
                            # cross-partition store into attnT rows 64-127.
                            qsl = slice(qj * 512, (qj + 1) * 512)
                            # previous strip's projection: issued first so its
                            # psum-freeing copies sit ahead of the normalize
                            # chain in the DVE queue
                            for _ in range(PROJ_PER_PAIR[qi][pr]):
                                if proj_fifo:
                                    t_, c_ = proj_fifo.pop(0)
                                    proj_chain(t_, c_, ps_y)
                            if qi == 0:
                                # deferred last-strip qk projections: PE
                                # filler for the activation-bound pairs
                                x3h, x3l = xts3
                                ts3 = T // SW - 1
                                for ct in (2 * pr, 2 * pr + 1):
                                    csl = slice(ct * P, (ct + 1) * P)
                                    psq = ps_qd.tile([P, SW], F32,
                                                     tag="psqd")
                                    for wq_, xt_, first, last in (
                                        (wqkh_sb, x3h, True, False),
                                        (wqkl_sb, x3h, False, False),
                                        (wqkh_sb, x3l, False, True),
                                    ):
                                        for kp in range(NKP):
                                            s = slice(2 * kp, 2 * kp + 2)
                                            nc.tensor.matmul(
                                                psq[:], wq_[:, s, csl],
                                                xt_[:, s, :],
                                                start=(first and kp == 0),
                                                stop=(last
                                                      and kp == NKP - 1),
                                                perf_mode=DRM)
                                    nc.vector.tensor_scalar_add(
                                        qkT[:, ct,
                                            ts3 * SW:(ts3 + 1) * SW],
                                        psq[:], bqk[:, ct:ct + 1])
                            ounA = ounp.tile([EH, 512], F32, tag="oun")
                            if qi == NS - 1 and pr == 3:
                                # last pair: ACT is drained of exps; split the
                                # two psum copies across engines
                                nc.scalar.copy(ounA[:], psoA[:])
                            else:
                                nc.vector.tensor_copy(ounA[:], psoA[:])
                            ounB = ounp.tile([EH, 512], F32, tag="oun")
                            nc.vector.tensor_copy(ounB[:], psoB[:])
                            # cross-partition reciprocal p64 -> p0 (DVE allows
                            # partition remaps between aligned bases); the
                            # broadcast source must sit at partition 0 on hw
                            rcA = smallp.tile([1, 512], F32, tag="rc")
                            nc.vector.reciprocal(rcA[:], ounA[HD:EH, :])
                            rcB = smallp.tile([1, 512], F32, tag="rc")
                            nc.vector.reciprocal(rcB[:], ounB[HD:EH, :])
                            bcA = smallp.tile([HD, 512], F32, tag="bc")
                            nc.gpsimd.partition_broadcast(bcA[:], rcA[:])
                            bcB = smallp.tile([HD, 512], F32, tag="bc")
                            nc.gpsimd.partition_broadcast(bcB[:], rcB[:])
                            nc.vector.tensor_tensor(
                                attnT[0:HD, pr, qsl],
                                ounA[0:HD, :], bcA[:], MUL,
                            )
                            # odd head: DVE cross-partition write (0-63 ->
                            # 64-127), aligned partition bases
                            nc.vector.tensor_tensor(
                                attnT[HD:P, pr, qsl],
                                ounB[0:HD, :], bcB[:], MUL,
                            )
                        for t_ in range(4 * qj, 4 * qj + 4):
                            for c_ in range(2):
                                proj_fifo.append((t_, c_))
                    _stk.close()
                # last strip's projection with a wide psum pool (the
                # attention pools are closed by now) so its first three
                # accumulation steps overlap the final normalize chain
                with tc.tile_pool(name="ps_f", bufs=4, space="PSUM") as ps_f:
                    chains = []
                    for tt in range(4 * STRIP_ORDER[-1],
                                    4 * STRIP_ORDER[-1] + 4):
                        chains.append(
                            (tt, ps_f.tile([P, 1024], F32, tag="psf",
                                           name=f"psf_{tt}")))
                    # stage-major so every chain's first three accumulation
                    # steps can run while the last head pair normalizes; the
                    # last stage goes chain-major so each chain's copy + DMA
                    # launches as soon as its accumulation closes
                    for ct in range(3):
                        for tt, psy in chains:
                            for co in range(2):
                                nc.tensor.matmul(
                                    psy[:, co * 512:(co + 1) * 512],
                                    attnT[:, ct, tt * P:(tt + 1) * P],
                                    wp_sb[:, ct, co * 512:(co + 1) * 512],
                                    start=(ct == 0), stop=False,
                                )
                    for i, (tt, psy) in enumerate(chains):
                        for co in range(2):
                            nc.tensor.matmul(
                                psy[:, co * 512:(co + 1) * 512],
                                attnT[:, 3, tt * P:(tt + 1) * P],
                                wp_sb[:, 3, co * 512:(co + 1) * 512],
                                start=False, stop=True,
                            )
                        yt = ystagep.tile([P, 1024], BF16, tag="ytf")
                        if i % 2 == 0:
                            nc.scalar.copy(yt[:], psy[:])
                        else:
                            nc.vector.tensor_copy(yt[:], psy[:])
                        nc.sync.dma_start(y_r[:, tt, :], yt[:])
    nc.compile()
    return nc


def _host_consts():
    import ml_dtypes
    i_idx = np.arange(P, dtype=np.float32)[:, None]
    j_idx = np.arange(P, dtype=np.float32)[None, :]
    maskb = np.where(j_idx - i_idx >= 0, 0.0, -1e30).astype(ml_dtypes.bfloat16)
    ident = np.eye(P, dtype=ml_dtypes.bfloat16)
    return ident, maskb


def make_in_maps(x, w_attn, b_attn, w_proj):
    import ml_dtypes
    bf16 = ml_dtypes.bfloat16
    f8 = ml_dtypes.float8_e4m3
    ident, maskb = _host_consts()
    in_maps = []
    for c in range(8):
        b = c // 2
        h0 = (c % 2) * HPC
        qcols = slice(h0 * HD, h0 * HD + CL)
        kcols = slice(C + h0 * HD, C + h0 * HD + CL)
        vcols = slice(2 * C + h0 * HD, 2 * C + h0 * HD + CL)
        wqk = np.concatenate([w_attn[:, qcols], w_attn[:, kcols]],
                             axis=1) * np.float32(WS)
        wv = w_attn[:, vcols] * np.float32(WS)
        bqk = (np.concatenate([b_attn[qcols], b_attn[kcols]])
               * np.float32(WS)).reshape(8, P).T
        xt = np.ascontiguousarray(x[b].T)
        xth = xt.astype(f8)
        xtl = (xt - xth.astype(np.float32)).astype(f8)
        wqkh = wqk.astype(f8)
        wqkl = (wqk - wqkh.astype(np.float32)).astype(f8)
        wvh = wv.astype(f8)
        wvl = (wv - wvh.astype(np.float32)).astype(f8)
        in_maps.append({
            "xth": xth, "xtl": np.ascontiguousarray(xtl),
            "wqkh": np.ascontiguousarray(wqkh),
            "wqkl": np.ascontiguousarray(wqkl),
            "wvh": np.ascontiguousarray(wvh),
            "wvl": np.ascontiguousarray(wvl),
            "wp": np.ascontiguousarray(
                (w_proj[h0 * HD:h0 * HD + CL, :]
                 / np.float32(WS)).astype(bf16)),
            "bqk": np.ascontiguousarray(bqk),
            "ident": ident,
            "maskb": maskb,
        })
    return in_maps


def _get_runner():
    """Build the SPMD executor once: a cached jax.jit over 8 cores.

    Mirrors bass2jax.run_bass_via_pjrt but hoists the jit so repeated
    kernel() calls reuse the compiled executable.
    """
    if "runner" in _CACHED:
        return _CACHED["runner"]
    import jax
    import jax.numpy as jnp
    from jax.sharding import Mesh, PartitionSpec
    from jax.experimental.shard_map import shard_map
    from concourse import bass2jax
    import concourse.mybir as mybir_

    nc = _CACHED.get("nc")
    if nc is None:
        nc = _CACHED["nc"] = build_nc()
    bass2jax.install_neuronx_cc_hook()

    partition_name = (nc.partition_id_tensor.name
                      if nc.partition_id_tensor else None)
    in_names, out_names, out_avals, zero_shapes = [], [], [], []
    for alloc in nc.m.functions[0].allocations:
        if not isinstance(alloc, mybir_.MemoryLocationSet):
            continue
        name = alloc.memorylocations[0].name
        if alloc.kind == "ExternalInput":
            if name != partition_name:
                in_names.append(name)
        elif alloc.kind == "ExternalOutput":
            shape = tuple(alloc.tensor_shape)
            dtype = mybir_.dt.np(alloc.dtype)
            out_names.append(name)
            out_avals.append(jax.core.ShapedArray(shape, dtype))
            zero_shapes.append((shape, dtype))
    n_params = len(in_names)
    n_outs = len(out_names)
    all_names = in_names + out_names
    if partition_name is not None:
        all_names = all_names + [partition_name]

    def _body(*args):
        operands = list(args)
        if partition_name is not None:
            operands.append(bass2jax.partition_id_tensor())
        outs = bass2jax._bass_exec_p.bind(
            *operands,
            out_avals=tuple(out_avals),
            in_names=tuple(all_names),
            out_names=tuple(out_names),
            lowering_input_output_aliases=(),
            sim_require_finite=True,
            sim_require_nnan=True,
            nc=nc,
        )
        return tuple(outs)

    devices = jax.devices()[:8]
    mesh = Mesh(np.asarray(devices), ("core",))
    in_specs = (PartitionSpec("core"),) * (n_params + n_outs)
    out_specs = (PartitionSpec("core"),) * n_outs
    donate = tuple(range(n_params, n_params + n_outs))
    sharded = jax.jit(
        shard_map(_body, mesh=mesh, in_specs=in_specs, out_specs=out_specs,
                  check_rep=False),
        donate_argnums=donate, keep_unused=True,
    )

    def run(in_maps):
        concat_in = [
            np.concatenate([np.asarray(in_maps[c][nm]) for c in range(8)],
                           axis=0)
            for nm in in_names
        ]
        concat_zeros = [
            np.zeros((8 * s[0], *s[1:]), dt) for (s, dt) in zero_shapes
        ]
        out_arrs = sharded(*concat_in, *concat_zeros)
        return [
            {nm: np.asarray(out_arrs[i]).reshape(8, *out_avals[i].shape)[c]
             for i, nm in enumerate(out_names)}
            for c in range(8)
        ]

    _CACHED["runner"] = run
    return run


def kernel(x, w_attn, b_attn, w_proj, b_proj):
    x = np.asarray(x, dtype=np.float32)
    w_attn = np.asarray(w_attn, dtype=np.float32)
    b_attn = np.asarray(b_attn, dtype=np.float32)
    w_proj = np.asarray(w_proj, dtype=np.float32)
    b_proj = np.asarray(b_proj, dtype=np.float32)

    in_maps = make_in_maps(x, w_attn, b_attn, w_proj)
    try:
        run = _get_runner()
        results = run(in_maps)
    except Exception:
        # fallback: the stock SPMD runner (slower per call, same result)
        if "nc" not in _CACHED:
            _CACHED["nc"] = build_nc()
        res = run_bass_kernel_spmd(
            _CACHED["nc"], in_maps, core_ids=list(range(8)))
        results = res.results

    # v-bias contribution: probs rows sum to 1, so attn += 1 * b_v^T, and
    # (1 b_v^T) @ w_proj = row vector b_v @ w_proj added to every position.
    extra = b_attn[2 * C:] @ w_proj + b_proj  # [C]
    out = np.empty((B, T, C), dtype=np.float32)
    for b in range(B):
        out[b] = (results[2 * b]["y"].astype(np.float32)
                  + results[2 * b + 1]["y"].astype(np.float32) + extra)
    return out

